# revision 1
# baseline (speedup 1.0000x reference)
"""Self-contained 8-core Trainium2 Bass kernel for nn_MultiHeadAttention.

Full (unsharded) inputs in, full output out. Sharding: core c handles
batch b = c // 2, query-half h = c % 2 (1024 queries). K/V projections for a
batch are computed redundantly on the 2 cores sharing it -> zero collectives,
disjoint outputs.

All matmuls run in float32r (TF32-like, ~1.6e-4 rel err, 4x fp32 throughput).
"""

import ml_dtypes
import numpy as np

import concourse.bass as bass
import concourse.mybir as mybir
from concourse import bacc
from concourse.tile import TileContext
from concourse.bass_utils import run_bass_kernel_spmd

F32 = mybir.dt.float32
F32R = mybir.dt.float32r
BF16 = mybir.dt.bfloat16
ACT = mybir.ActivationFunctionType

B, S, D = 4, 2048, 1024
H, DK = 16, 64
SQ = S // 2            # queries per core
P = 128
NCORES = 8
FC = D // P            # 8 feature chunks (contraction)
OFC = D // P           # 8 output-feature chunks
PAIRS = H // 2         # 8 head pairs (2 heads = 128 partitions)
NKT = S // P           # 16 key tiles of 128 tokens
QTW = 512              # q tile width
NQT = SQ // QTW        # 2
EB = 2                 # key tiles per exp batch (2 psum banks per scores tile)
SCALE = 1.0 / np.sqrt(np.float32(DK))


def build_nc():
    nc = bacc.Bacc()

    xq = nc.declare_dram_parameter("xq_t", [D, SQ], F32R, isOutput=False)
    xk = nc.declare_dram_parameter("xk_t", [D, S], F32R, isOutput=False)
    xv = nc.declare_dram_parameter("xv_t", [D, S], F32R, isOutput=False)
    wq = nc.declare_dram_parameter("wq_t", [D, D], F32R, isOutput=False)
    wk = nc.declare_dram_parameter("wk_t", [D, D], F32R, isOutput=False)
    wv = nc.declare_dram_parameter("wv_t", [D, D], F32R, isOutput=False)
    wo = nc.declare_dram_parameter("wo_t", [D, D], BF16, isOutput=False)
    bq = nc.declare_dram_parameter("b_q_r", [P, OFC], F32, isOutput=False)
    bk = nc.declare_dram_parameter("b_k_r", [P, OFC], F32, isOutput=False)
    bo = nc.declare_dram_parameter("b_o_r", [P, OFC], F32, isOutput=False)
    ones_in = nc.declare_dram_parameter("ones_row", [1, P], F32R, isOutput=False)
    vones_in = nc.declare_dram_parameter("v_ones", [P, NKT, 2, 1], F32R,
                                         isOutput=False)
    out = nc.declare_dram_parameter("out_t", [D, SQ], F32, isOutput=True)

    kt_scr = nc.dram_tensor("kt_scr", [P, PAIRS, S], F32R)
    v_scr = nc.dram_tensor("v_scr", [P, PAIRS, NKT, 2, DK], F32R)

    with nc.allow_low_precision(reason="f32r compute"), TileContext(nc) as tc:
        with (
            tc.tile_pool(name="persist", bufs=1) as pers,
            tc.tile_pool(name="norm", bufs=2) as npool,
        ):
            qt_s = pers.tile([P, OFC, SQ], F32R, tag="qt")
            attn_t = pers.tile([P, PAIRS, SQ], BF16, tag="attnt")
            tbq = pers.tile([P, OFC], F32, tag="tbq")
            tbk = pers.tile([P, OFC], F32, tag="tbk")
            tbo = pers.tile([P, OFC], F32, tag="tbo")
            tones = pers.tile([1, P], F32R, tag="tones")
            vones_s = pers.tile([P, NKT, 2, 1], F32R, tag="vones")
            nc.sync.dma_start(tbq[:], bq[:])
            nc.sync.dma_start(tbk[:], bk[:])
            nc.sync.dma_start(tbo[:], bo[:])
            nc.sync.dma_start(tones[:], ones_in[:])
            nc.sync.dma_start(vones_s[:], vones_in[:])

            # ---------------- Stage A: projections ----------------
            # kpairpool is opened alongside stage A so the first head-pair's
            # K/V loads can overlap the stage-A tail.
            kpp_cm = tc.tile_pool(name="kpairpool", bufs=2)
            kpp = kpp_cm.__enter__()
            with (
                tc.tile_pool(name="wpool", bufs=2) as wpool,
                tc.tile_pool(name="xpool", bufs=2) as xpool,
                tc.tile_pool(name="kspool", bufs=3) as kspool,
                tc.tile_pool(name="apsum", bufs=4, space="PSUM") as apsum,
            ):
                def load_w(src, dt=F32R):
                    wt = wpool.tile([P, FC, D], dt, tag="w_full")
                    for fc in range(FC):
                        nc.sync.dma_start(wt[:, fc, :],
                                          src[fc * P:(fc + 1) * P, :])
                    return wt

                def load_x(src, tt, dt=F32R):
                    xts = []
                    for fc in range(FC):
                        xt = xpool.tile([P, QTW], dt, tag=f"x{fc}")
                        nc.sync.dma_start(
                            xt[:], src[fc * P:(fc + 1) * P,
                                       tt * QTW:(tt + 1) * QTW])
                        xts.append(xt)
                    return xts

                # --- Q projection: QT[of, t] = Wq @ xq ---
                # Interleave W and x DMAs, and split each W row-chunk so the
                # first matmul's 128 columns arrive in ~1us, not after the
                # whole W load.
                wtq = wpool.tile([P, FC, D], F32R, tag="w_full")
                xq_tiles = [[], []]
                for fc in range(FC):
                    nc.sync.dma_start(wtq[:, fc, 0:P],
                                      wq[fc * P:(fc + 1) * P, 0:P])
                    for tt in range(NQT):
                        xt = xpool.tile([P, QTW], F32R, tag=f"x{fc}",
                                        name=f"xq{fc}_{tt}")
                        nc.sync.dma_start(
                            xt[:], xq[fc * P:(fc + 1) * P,
                                      tt * QTW:(tt + 1) * QTW])
                        xq_tiles[tt].append(xt)
                    nc.sync.dma_start(wtq[:, fc, P:],
                                      wq[fc * P:(fc + 1) * P, P:])
                for tt in range(NQT):
                    xts = xq_tiles[tt]
                    for ofc in range(OFC):
                        ps = apsum.tile([P, QTW], F32, tag="acc")
                        for fc in range(FC):
                            nc.tensor.matmul(
                                ps[:],
                                wtq[:, fc, ofc * P:(ofc + 1) * P],
                                xts[fc][:],
                                start=(fc == 0), stop=(fc == FC - 1),
                            )
                        nc.vector.tensor_scalar_add(
                            qt_s[:, ofc, tt * QTW:(tt + 1) * QTW], ps[:],
                            tbq[:, ofc:ofc + 1])

                # --- K projection -> kt_scr[of, t] (DRAM bounce) ---
                wtk = load_w(wk)
                for tt in range(S // QTW):
                    xts = load_x(xk, tt)
                    for ofc in range(OFC):
                        ps = apsum.tile([P, QTW], F32, tag="acc")
                        for fc in range(FC):
                            nc.tensor.matmul(
                                ps[:],
                                wtk[:, fc, ofc * P:(ofc + 1) * P],
                                xts[fc][:],
                                start=(fc == 0), stop=(fc == FC - 1),
                            )
                        ks = kspool.tile([P, QTW], F32R, tag="ks")
                        nc.vector.tensor_scalar_add(ks[:], ps[:],
                                                    tbk[:, ofc:ofc + 1])
                        nc.sync.dma_start(
                            kt_scr[:, ofc, tt * QTW:(tt + 1) * QTW], ks[:])

                # --- V projection: V[t, of] (natural), no bias (folded) ---
                wtv = load_w(wv)
                for ktg in range(4):  # groups of 4 key tiles (512 tokens)
                    xts = load_x(xv, ktg)
                    for ki in range(4):
                        kt = ktg * 4 + ki
                        for half in range(2):
                            ps = apsum.tile([P, QTW], F32, tag="acc")
                            for fc in range(FC):
                                nc.tensor.matmul(
                                    ps[:],
                                    xts[fc][:, ki * P:(ki + 1) * P],
                                    wtv[:, fc, half * 512:(half + 1) * 512],
                                    start=(fc == 0), stop=(fc == FC - 1),
                                )
                            # scatter 512 of-cols (= 4 pairs x 2 heads x 64)
                            # to DRAM scratch via SBUF staging
                            vs = kspool.tile([P, QTW], F32R, tag="vs")
                            nc.vector.tensor_copy(vs[:], ps[:])
                            nc.sync.dma_start(
                                v_scr[:, half * 4:(half + 1) * 4, kt, :, :],
                                vs[:].rearrange(
                                    "p (c h2 d) -> p c h2 d", c=4, h2=2),
                            )

            # ---------------- Stage B: attention, head-pairs row-tiled ----
            with tc.tile_pool(name="wopool", bufs=1) as wop:
                # prefetch W_out (bf16) for stage C while attention runs
                wto = wop.tile([P, FC, D], BF16, tag="wo_full")
                for fc in range(FC):
                    nc.sync.dma_start(wto[:, fc, :], wo[fc * P:(fc + 1) * P, :])

                with (
                    tc.tile_pool(name="ppool", bufs=2) as ppool,
                    tc.tile_pool(name="utpool", bufs=2) as utp,
                    tc.tile_pool(name="bpsum", bufs=1, space="PSUM") as bpsum,
                ):
                    self_attention(nc, tc, kpp, ppool, utp, bpsum, npool,
                                   kt_scr, v_scr, qt_s, attn_t, vones_s, tones)

                # ---------- Stage C: out projection (transposed) ----------
                with (
                    tc.tile_pool(name="opool", bufs=3) as opool,
                    tc.tile_pool(name="cpsum", bufs=3, space="PSUM") as cpsum,
                ):
                    for tt in range(NQT):
                        for ofc in range(OFC):
                            ps = cpsum.tile([P, QTW], F32, tag="oacc")
                            for c in range(PAIRS):
                                nc.tensor.matmul(
                                    ps[:],
                                    wto[:, c, ofc * P:(ofc + 1) * P],
                                    attn_t[:, c, tt * QTW:(tt + 1) * QTW],
                                    start=(c == 0), stop=(c == PAIRS - 1),
                                )
                            osb = opool.tile([P, QTW], F32, tag="osb")
                            nc.vector.tensor_scalar_add(osb[:], ps[:],
                                                        tbo[:, ofc:ofc + 1])
                            nc.sync.dma_start(
                                out[ofc * P:(ofc + 1) * P,
                                    tt * QTW:(tt + 1) * QTW], osb[:])

            kpp_cm.__exit__(None, None, None)

    nc.finalize()
    return nc


def self_attention(nc, tc, kpp, ppool, utp, bpsum, npool, kt_scr, v_scr,
       qt_s, attn_t, vones_s, tones):
    for c in range(PAIRS):
        kpair = kpp.tile([P, S], F32R, tag="kpair")
        nc.sync.dma_start(kpair[:], kt_scr[:, c, :])
        vpair = kpp.tile([P, NKT, 2, DK + 1], F32R, tag="vpair")
        nc.sync.dma_start(vpair[:, :, :, 0:DK], v_scr[:, c])
        nc.vector.tensor_copy(vpair[:, :, :, DK:DK + 1], vones_s[:])
        for qt in range(NQT):
            qsl = slice(qt * QTW, (qt + 1) * QTW)
            # both heads of the pair: scores via row-tiled
            # concurrent K=64 matmuls (rows 0-63 / 64-127)
            # P-tiles split in two kt-halves for finer exp/PV pipelining
            pts = [[ppool.tile([P, NKT // 2, QTW], F32R,
                               tag=f"pt{h2}{hf}", name=f"pt{h2}{hf}", bufs=1)
                    for hf in range(2)]
                   for h2 in range(2)]
            for ktb in range(NKT // EB):
                sps = [bpsum.tile([P, EB * QTW], F32,
                                  tag=f"sc{h2}", name=f"sc{h2}")
                       for h2 in range(2)]
                for e in range(EB):
                    kt = ktb * EB + e
                    for h2 in range(2):
                        base = h2 * DK
                        nc.tensor.matmul(
                            sps[h2][:, e * QTW:(e + 1) * QTW],
                            kpair[base:base + DK,
                                  kt * P:(kt + 1) * P],
                            qt_s[base:base + DK, c, qsl],
                            start=True, stop=True,
                            tile_position=(base, 0),
                        )
                for h2 in range(2):
                    kb0 = ktb * EB
                    hf, off = divmod(kb0, NKT // 2)
                    nc.scalar.activation(
                        pts[h2][hf][:, off:off + EB, :],
                        sps[h2][:], ACT.Exp, scale=float(SCALE))
            for h2 in range(2):
                base = h2 * DK
                ut = bpsum.tile([DK + 1, QTW], F32, tag="ut", bufs=2)
                for kt in range(NKT):
                    hf, koff = divmod(kt, NKT // 2)
                    nc.tensor.matmul(
                        ut[:],
                        vpair[:, kt, h2, :],
                        pts[h2][hf][:, koff, :],
                        start=(kt == 0), stop=(kt == NKT - 1),
                    )
                recip = npool.tile([1, QTW], F32R, tag="recip")
                nc.vector.reciprocal(recip[:], ut[DK:DK + 1, :])
                bc = bpsum.tile([P, QTW], F32, tag="bc", bufs=2)
                nc.tensor.matmul(bc[:], tones[:], recip[:],
                                 start=True, stop=True)
                uts = utp.tile([DK, QTW], F32, tag="uts")
                nc.vector.tensor_copy(uts[:], ut[0:DK, :])
                nc.vector.tensor_mul(
                    attn_t[base:base + DK, c, qsl],
                    uts[:], bc[0:DK, :])
    return nc


def _prep_host(query, key, value, W_q, b_q, W_k, b_k, W_v, b_v, W_out, b_out):
    """Host-side layout prep (transposes / bias folding). No math beyond the
    b_v fold, which is a 1024x1024 matvec."""
    f32 = np.float32
    query = np.asarray(query, f32)
    key = np.asarray(key, f32)
    value = np.asarray(value, f32)
    W_q = np.asarray(W_q, f32)
    W_k = np.asarray(W_k, f32)
    W_v = np.asarray(W_v, f32)
    W_out = np.asarray(W_out, f32)
    b_q = np.asarray(b_q, f32)
    b_k = np.asarray(b_k, f32)
    b_v = np.asarray(b_v, f32)
    b_out = np.asarray(b_out, f32)

    common = {
        "wq_t": np.ascontiguousarray(W_q.T),
        "wk_t": np.ascontiguousarray(W_k.T),
        "wv_t": np.ascontiguousarray(W_v.T),
        "wo_t": np.ascontiguousarray(W_out.T).astype(ml_dtypes.bfloat16),
        "b_q_r": np.ascontiguousarray(b_q.reshape(OFC, P).T),
        "b_k_r": np.ascontiguousarray(b_k.reshape(OFC, P).T),
        "b_o_r": np.ascontiguousarray(
            (b_out + W_out @ b_v).reshape(OFC, P).T.astype(f32)),
        "ones_row": np.ones((1, P), f32),
        "v_ones": np.ones((P, NKT, 2, 1), f32),
    }
    in_maps = []
    for c in range(NCORES):
        b, hf = divmod(c, 2)
        m = dict(common)
        m["xq_t"] = np.ascontiguousarray(
            query[b, hf * SQ:(hf + 1) * SQ, :].T)
        m["xk_t"] = np.ascontiguousarray(key[b].T)
        m["xv_t"] = np.ascontiguousarray(value[b].T)
        in_maps.append(m)
    return in_maps


_NC_CACHE = {}


def get_nc():
    if "nc" not in _NC_CACHE:
        _NC_CACHE["nc"] = build_nc()
    return _NC_CACHE["nc"]


def get_runner():
    """Build (once) a cached jitted SPMD callable over 8 cores.

    Mirrors concourse.bass2jax.run_bass_via_pjrt's multi-core path, but keeps
    the jitted function so repeated calls don't recompile the NEFF.
    """
    if "runner" in _NC_CACHE:
        return _NC_CACHE["runner"]

    import jax
    from jax.experimental.shard_map import shard_map
    from jax.sharding import Mesh, PartitionSpec

    from concourse import bass2jax

    nc = get_nc()
    bass2jax.install_neuronx_cc_hook()
    partition_name = (
        nc.partition_id_tensor.name if nc.partition_id_tensor else None
    )

    in_names, out_names, out_avals, zero_shapes = [], [], [], []
    for alloc in nc.m.functions[0].allocations:
        if not isinstance(alloc, mybir.MemoryLocationSet):
            continue
        name = alloc.memorylocations[0].name
        if alloc.kind == "ExternalInput":
            if name != partition_name:
                in_names.append(name)
        elif alloc.kind == "ExternalOutput":
            shape = tuple(alloc.tensor_shape)
            dtype = mybir.dt.np(alloc.dtype)
            out_names.append(name)
            out_avals.append(jax.core.ShapedArray(shape, dtype))
            zero_shapes.append((shape, dtype))
    n_params = len(in_names)
    n_outs = len(out_names)
    all_names = in_names + out_names
    if partition_name is not None:
        all_names = all_names + [partition_name]
    donate = tuple(range(n_params, n_params + n_outs))

    def _body(*args):
        operands = list(args)
        if partition_name is not None:
            operands.append(bass2jax.partition_id_tensor())
        outs = bass2jax._bass_exec_p.bind(
            *operands,
            out_avals=tuple(out_avals),
            in_names=tuple(all_names),
            out_names=tuple(out_names),
            lowering_input_output_aliases=(),
            sim_require_finite=True,
            sim_require_nnan=True,
            nc=nc,
        )
        return tuple(outs)

    devices = jax.devices()[:NCORES]
    mesh = Mesh(np.asarray(devices), ("core",))
    in_specs = (PartitionSpec("core"),) * (n_params + n_outs)
    out_specs = (PartitionSpec("core"),) * n_outs
    sharded = jax.jit(
        shard_map(_body, mesh=mesh, in_specs=in_specs, out_specs=out_specs,
                  check_rep=False),
        donate_argnums=donate,
        keep_unused=True,
    )

    def run(in_maps):
        concat_in = [
            np.concatenate([np.asarray(in_maps[c][n]) for c in range(NCORES)],
                           axis=0)
            for n in in_names
        ]
        zeros = [np.zeros((NCORES * s[0], *s[1:]), d) for s, d in zero_shapes]
        out_arrs = sharded(*concat_in, *zeros)
        return [
            {
                n: np.asarray(out_arrs[i]).reshape(
                    NCORES, *out_avals[i].shape)[c]
                for i, n in enumerate(out_names)
            }
            for c in range(NCORES)
        ]

    runner = {
        "run": run,
        "sharded": sharded,
        "in_names": in_names,
        "out_names": out_names,
        "out_avals": out_avals,
        "zero_shapes": zero_shapes,
        "mesh": mesh,
    }
    _NC_CACHE["runner"] = runner
    return runner


def kernel(**inputs) -> np.ndarray:
    in_maps = _prep_host(**inputs)
    results = get_runner()["run"](in_maps)
    out = np.empty((B, S, D), np.float32)
    for c in range(NCORES):
        b, hf = divmod(c, 2)
        out[b, hf * SQ:(hf + 1) * SQ, :] = results[c]["out_t"].T
    return out



# revision 7
# speedup vs baseline: 1.0768x; 1.0768x over previous
"""Self-contained 8-core Trainium2 Bass kernel for nn_MultiHeadAttention.

Full (unsharded) inputs in, full output out. Sharding: core c handles
batch b = c // 2, query-half h = c % 2 (1024 queries). K/V projections for a
batch are computed redundantly on the 2 cores sharing it -> zero collectives,
disjoint outputs.

Pipeline (v3):
 - All loads host-packed so each tensor arrives in 1-8 large DMAs.
 - K/V SBUF-resident bf16; no DRAM scratch round trip.
 - PV computed transposed (stationary = probs [keys, q], moving = V[keys, dk]
   + fused ones column): full 128-wide PE output, denominator lands on the
   same partition as its row. PSUM accumulators are memset once and all PV
   matmuls accumulate (start=True would reset the whole shared bank).
 - K projection per head-pair and the previous unit's normalize/transpose
   chains are woven into each attention unit's score/exp slots, keeping the
   Activation engine's exp stream dense.
 - Out-projection for the first query half woven into the last pair.
"""

import ml_dtypes
import numpy as np

import concourse.bass as bass
import concourse.mybir as mybir
from concourse import bacc
from concourse.tile import TileContext
from concourse.bass_utils import run_bass_kernel_spmd

F32 = mybir.dt.float32
F32R = mybir.dt.float32r
BF16 = mybir.dt.bfloat16
ACT = mybir.ActivationFunctionType

B, S, D = 4, 2048, 1024
H, DK = 16, 64
SQ = S // 2            # queries per core
P = 128
NCORES = 8
FC = D // P            # 8 contraction chunks
OFC = D // P           # 8 output-feature chunks
PAIRS = H // 2         # 8 head pairs (2 heads = 128 partitions)
NKT = S // P           # 16 key tiles of 128 tokens
QTW = 512              # q tile width
NQT = SQ // QTW        # 2
NQB = QTW // P         # 4 q-blocks of 128 per q tile
VKG = 256              # v-projection key group
SCALE = 1.0 / np.sqrt(np.float32(DK))
LAG = 2                # PV lags scores/exp by this many kt-pair slots


def build_nc():
    nc = bacc.Bacc()

    xq = nc.declare_dram_parameter("xq_p", [P, FC, SQ], F32R, isOutput=False)
    xkb = nc.declare_dram_parameter("xk_p", [P, FC, S], BF16, isOutput=False)
    xv = nc.declare_dram_parameter("xv_p", [S // VKG, P, FC, VKG], F32R,
                                   isOutput=False)
    wq = nc.declare_dram_parameter("wq_p", [OFC, P, FC, P], F32R,
                                   isOutput=False)
    wkb = nc.declare_dram_parameter("wk_p", [P, FC, D], BF16, isOutput=False)
    wv = nc.declare_dram_parameter("wv_p", [P, FC, D], F32R, isOutput=False)
    wo = nc.declare_dram_parameter("wo_p", [P, PAIRS, D], BF16, isOutput=False)
    bq = nc.declare_dram_parameter("b_q_r", [P, OFC], F32, isOutput=False)
    bk = nc.declare_dram_parameter("b_k_r", [P, OFC], F32, isOutput=False)
    bo = nc.declare_dram_parameter("b_o_r", [P, OFC], F32, isOutput=False)
    idn = nc.declare_dram_parameter("ident", [P, P], BF16, isOutput=False)
    out = nc.declare_dram_parameter("out_t", [D, SQ], F32, isOutput=True)

    with nc.allow_low_precision(reason="bf16 attention"), TileContext(nc) as tc:
        with tc.tile_pool(name="pers", bufs=1) as pers:
            xk_s = pers.tile([P, FC, S], BF16, tag="xk")
            wk_s = pers.tile([P, FC, D], BF16, tag="wk")
            qt_s = pers.tile([P, OFC, SQ], BF16, tag="qt")
            v_all = pers.tile([P, NKT, H, DK + 1], BF16, tag="vall")
            tbq = pers.tile([P, OFC], F32, tag="tbq")
            tbk = pers.tile([P, OFC], F32, tag="tbk")
            tbo = pers.tile([P, OFC], F32, tag="tbo")
            ident = pers.tile([P, P], BF16, tag="ident")
            nc.sync.dma_start(tbq[:], bq[:])
            nc.sync.dma_start(tbk[:], bk[:])
            nc.sync.dma_start(tbo[:], bo[:])
            nc.sync.dma_start(ident[:], idn[:])
            nc.vector.memset(v_all[:, :, :, DK:DK + 1], 1.0)

            # ---------------- Stage A: Q + V projections ----------------
            # Pools opened together so V loads prefetch during Q compute.
            with (
                tc.tile_pool(name="xqpool", bufs=1) as xqp,
                tc.tile_pool(name="wqpool", bufs=2) as wqp,
                tc.tile_pool(name="wvpool", bufs=1) as wvp,
                tc.tile_pool(name="xvpool", bufs=2) as xvp,
                tc.tile_pool(name="apsum", bufs=4, space="PSUM") as apsum,
            ):
                xq_t = xqp.tile([P, FC, SQ], F32R, tag="xq")

                def load_wq(ofc):
                    wqt = wqp.tile([P, FC, P], F32R, tag="wq",
                                   name=f"wq{ofc}")
                    nc.sync.dma_start(wqt[:], wq[ofc])
                    return wqt

                def load_xv(g):
                    xvt = xvp.tile([P, FC, VKG], F32R, tag="xv",
                                   name=f"xv{g}")
                    nc.sync.dma_start(xvt[:], xv[g])
                    return xvt

                wq_cur = load_wq(0)
                nc.sync.dma_start(xq_t[:], xq[:])
                wvt = wvp.tile([P, FC, D], F32R, tag="wv")
                nc.sync.dma_start(wvt[:], wv[:])
                xv_cur = load_xv(0)

                # K/V-pair inputs for stage B arrive during stage A compute
                nc.sync.dma_start(xk_s[:], xkb[:])
                nc.sync.dma_start(wk_s[:], wkb[:])

                def qproj(ofc, wqt):
                    for qt in range(NQT):
                        qsl = slice(qt * QTW, (qt + 1) * QTW)
                        ps = apsum.tile([P, QTW], F32, tag="aps")
                        for fc in range(FC):
                            nc.tensor.matmul(
                                ps[:], wqt[:, fc, :], xq_t[:, fc, qsl],
                                start=(fc == 0), stop=(fc == FC - 1))
                        nc.vector.tensor_scalar_add(
                            qt_s[:, ofc, qsl], ps[:], tbq[:, ofc:ofc + 1])

                def vproj(g, xvt):
                    for ki in range(VKG // P):
                        kt = (g * VKG) // P + ki
                        for half in range(2):
                            ps = apsum.tile([P, QTW], F32, tag="aps")
                            for fc in range(FC):
                                nc.tensor.matmul(
                                    ps[:],
                                    xvt[:, fc, ki * P:(ki + 1) * P],
                                    wvt[:, fc, half * QTW:(half + 1) * QTW],
                                    start=(fc == 0), stop=(fc == FC - 1))
                            nc.vector.tensor_copy(
                                v_all[:, kt, half * 8:(half + 1) * 8, 0:DK],
                                ps[:].rearrange("p (h d) -> p h d", h=8))

                for ofc in range(OFC):
                    wq_nxt = load_wq(ofc + 1) if ofc + 1 < OFC else None
                    qproj(ofc, wq_cur)
                    wq_cur = wq_nxt
                    xv_nxt = load_xv(ofc + 1) if ofc + 1 < OFC else None
                    vproj(ofc, xv_cur)
                    xv_cur = xv_nxt

            # ---------------- Stage B: woven attention ----------------
            with (
                tc.tile_pool(name="kattn", bufs=1) as katp,
                tc.tile_pool(name="ptspool", bufs=4) as ptsp,
                tc.tile_pool(name="arawpool", bufs=2) as arawp,
                tc.tile_pool(name="npool", bufs=2) as npool,
                tc.tile_pool(name="opool", bufs=2) as opool,
                tc.tile_pool(name="spsum", bufs=1, space="PSUM") as spsum,
                tc.tile_pool(name="acpsum", bufs=1, space="PSUM") as acpsum,
                tc.tile_pool(name="auxpsum", bufs=2, space="PSUM") as auxp,
            ):
                k_all = katp.tile([P, PAIRS, S], BF16, tag="kall")
                attn_t = katp.tile([P, PAIRS, SQ], BF16, tag="attnt")
                wto = katp.tile([P, PAIRS, D], BF16, tag="wo")
                nc.sync.dma_start(wto[:], wo[:])

                def kproj_chunk(c, tt):
                    ps = auxp.tile([P, QTW], F32, tag="aux", name=f"kp{c}_{tt}")
                    tsl = slice(tt * QTW, (tt + 1) * QTW)
                    for fc in range(FC):
                        nc.tensor.matmul(
                            ps[:], wk_s[:, fc, c * P:(c + 1) * P],
                            xk_s[:, fc, tsl],
                            start=(fc == 0), stop=(fc == FC - 1))
                    nc.vector.tensor_scalar_add(
                        k_all[:, c, tsl], ps[:], tbk[:, c:c + 1])

                def c_chunk(qt, ofc):
                    qsl = slice(qt * QTW, (qt + 1) * QTW)
                    ps = auxp.tile([P, QTW], F32, tag="aux",
                                   name=f"cc{qt}_{ofc}")
                    for cc in range(PAIRS):
                        nc.tensor.matmul(
                            ps[:], wto[:, cc, ofc * P:(ofc + 1) * P],
                            attn_t[:, cc, qsl],
                            start=(cc == 0), stop=(cc == PAIRS - 1))
                    osb = opool.tile([P, QTW], F32, tag="osb")
                    nc.vector.tensor_scalar_add(osb[:], ps[:],
                                                tbo[:, ofc:ofc + 1])
                    nc.sync.dma_start(
                        out[ofc * P:(ofc + 1) * P, qsl], osb[:])

                def attn_unit(c, qt, fillers, pending):
                    """Emit one (pair, q-tile) attention unit. `pending` are
                    the previous unit's normalize/transpose chains, drained in
                    the early slots; returns this unit's chains."""
                    qsl = slice(qt * QTW, (qt + 1) * QTW)
                    accs = [acpsum.tile([P, NQB, P], F32, tag=f"acc{h2}",
                                        name=f"acc{c}_{qt}_{h2}")
                            for h2 in range(2)]
                    for h2 in range(2):
                        nc.vector.memset(accs[h2][:], 0.0)
                    ptss = {}

                    def scores_exp(i):
                        for h2 in range(2):
                            base = h2 * DK
                            sps = spsum.tile(
                                [P, 2, QTW], F32, tag=f"sps{h2}",
                                name=f"sps{c}_{qt}_{i}_{h2}")
                            for e in range(2):
                                kt = 2 * i + e
                                nc.tensor.matmul(
                                    sps[:, e, :],
                                    k_all[base:base + DK, c,
                                          kt * P:(kt + 1) * P],
                                    qt_s[base:base + DK, c, qsl],
                                    start=True, stop=True,
                                    tile_position=(base, 0))
                            pt = ptsp.tile([P, 2, QTW], BF16, tag=f"pt{h2}",
                                           name=f"pt{c}_{qt}_{i}_{h2}")
                            nc.scalar.activation(pt[:], sps[:], ACT.Exp,
                                                 scale=float(SCALE))
                            ptss[(i, h2)] = pt

                    def pv(i):
                        for h2 in range(2):
                            for e in range(2):
                                kt = 2 * i + e
                                for qb in range(NQB):
                                    nc.tensor.matmul(
                                        accs[h2][:, qb, 0:DK + 1],
                                        ptss[(i, h2)][:, e,
                                                      qb * P:(qb + 1) * P],
                                        v_all[:, kt, 2 * c + h2, :],
                                        start=False, stop=(kt == NKT - 1),
                                        skip_group_check=True)

                    pops = [3, 3, 2, 0, 0, 0, 0, 0]
                    for i in range(NKT // 2):
                        scores_exp(i)
                        for _ in range(pops[i]):
                            if pending:
                                pending.pop(0)()
                        if i >= LAG:
                            pv(i - LAG)
                        if fillers and i >= 3:
                            fillers.pop(0)()
                    for i in range(NKT // 2 - LAG, NKT // 2):
                        pv(i)
                    while fillers:
                        fillers.pop(0)()

                    # spill accumulators to SBUF so the PSUM bank frees
                    # immediately and the chains can drain next unit
                    araws = []
                    for h2 in range(2):
                        araw = arawp.tile([P, NQB, DK + 1], F32, tag="araw",
                                          name=f"araw{c}_{qt}_{h2}")
                        nc.vector.tensor_copy(araw[:],
                                              accs[h2][:, :, 0:DK + 1])
                        araws.append(araw)

                    def make_post(h2, qb):
                        def post():
                            base = h2 * DK
                            araw = araws[h2]
                            recip = npool.tile([P, 1], F32, tag="recip")
                            nc.vector.reciprocal(recip[:],
                                                 araw[:, qb, DK:DK + 1])
                            anorm = npool.tile([P, DK], BF16, tag="anorm")
                            nc.vector.tensor_scalar_mul(
                                anorm[:], araw[:, qb, 0:DK], recip[:])
                            aux = auxp.tile([P, QTW], F32, tag="aux",
                                            name=f"tp{c}_{qt}_{h2}_{qb}")
                            tp = aux[0:DK, 0:DK].bitcast(BF16)
                            nc.tensor.transpose(tp, anorm[:], ident[:])
                            nc.vector.tensor_copy(
                                attn_t[base:base + DK, c,
                                       qt * QTW + qb * P:
                                       qt * QTW + (qb + 1) * P], tp)
                        return post

                    return [make_post(h2, qb)
                            for qb in range(NQB) for h2 in range(2)]

                for tt in range(4):
                    kproj_chunk(0, tt)
                pending = []
                for c in range(PAIRS):
                    for qt in range(NQT):
                        fillers = []
                        if c + 1 < PAIRS:
                            fillers = [
                                (lambda c_=c + 1, t_=t: kproj_chunk(c_, t_))
                                for t in (2 * qt, 2 * qt + 1)
                            ]
                        elif qt == 1:
                            fillers = [
                                (lambda o_=o: c_chunk(0, o_))
                                for o in range(OFC)
                            ]
                        pending = attn_unit(c, qt, fillers, pending)
                for p_ in pending:
                    p_()
                for ofc in range(OFC):
                    c_chunk(1, ofc)

    nc.finalize()
    return nc


def _prep_host(query, key, value, W_q, b_q, W_k, b_k, W_v, b_v, W_out, b_out):
    """Host-side layout prep (packing / transposes / bias folding). No math
    beyond the b_v fold, which is a 1024x1024 matvec."""
    f32 = np.float32
    bf16 = ml_dtypes.bfloat16
    query = np.asarray(query, f32)
    key = np.asarray(key, f32)
    value = np.asarray(value, f32)
    W_q = np.asarray(W_q, f32)
    W_k = np.asarray(W_k, f32)
    W_v = np.asarray(W_v, f32)
    W_out = np.asarray(W_out, f32)
    b_q = np.asarray(b_q, f32)
    b_k = np.asarray(b_k, f32)
    b_v = np.asarray(b_v, f32)
    b_out = np.asarray(b_out, f32)

    def pack_w(wt, dt):  # [D(in), D(of)] -> [P, FC, D(of)]
        return np.ascontiguousarray(
            wt.reshape(FC, P, D).transpose(1, 0, 2)).astype(dt)

    def pack_x(xt, dt, width, n):  # [D, T] -> [n, P, FC, width]
        return np.ascontiguousarray(
            xt.reshape(FC, P, n, width).transpose(2, 1, 0, 3)).astype(dt)

    common = {
        "wq_p": np.ascontiguousarray(
            W_q.T.reshape(FC, P, OFC, P).transpose(2, 1, 0, 3)),
        "wk_p": pack_w(W_k.T, bf16),
        "wv_p": pack_w(W_v.T, f32),
        "wo_p": pack_w(W_out.T, bf16),
        "b_q_r": np.ascontiguousarray(b_q.reshape(OFC, P).T),
        "b_k_r": np.ascontiguousarray(b_k.reshape(OFC, P).T),
        "b_o_r": np.ascontiguousarray(
            (b_out + W_out @ b_v).reshape(OFC, P).T.astype(f32)),
        "ident": np.eye(P, dtype=bf16),
    }
    in_maps = []
    for c in range(NCORES):
        b, hf = divmod(c, 2)
        m = dict(common)
        m["xq_p"] = pack_x(query[b, hf * SQ:(hf + 1) * SQ, :].T, f32,
                           SQ, 1)[0]
        m["xk_p"] = pack_x(key[b].T, bf16, S, 1)[0]
        m["xv_p"] = pack_x(value[b].T, f32, VKG, S // VKG)
        in_maps.append(m)
    return in_maps


_NC_CACHE = {}


def get_nc():
    if "nc" not in _NC_CACHE:
        _NC_CACHE["nc"] = build_nc()
    return _NC_CACHE["nc"]


def get_runner():
    """Build (once) a cached jitted SPMD callable over 8 cores.

    Mirrors concourse.bass2jax.run_bass_via_pjrt's multi-core path, but keeps
    the jitted function so repeated calls don't recompile the NEFF.
    """
    if "runner" in _NC_CACHE:
        return _NC_CACHE["runner"]

    import jax
    from jax.experimental.shard_map import shard_map
    from jax.sharding import Mesh, PartitionSpec

    from concourse import bass2jax

    nc = get_nc()
    bass2jax.install_neuronx_cc_hook()
    partition_name = (
        nc.partition_id_tensor.name if nc.partition_id_tensor else None
    )

    in_names, out_names, out_avals, zero_shapes = [], [], [], []
    for alloc in nc.m.functions[0].allocations:
        if not isinstance(alloc, mybir.MemoryLocationSet):
            continue
        name = alloc.memorylocations[0].name
        if alloc.kind == "ExternalInput":
            if name != partition_name:
                in_names.append(name)
        elif alloc.kind == "ExternalOutput":
            shape = tuple(alloc.tensor_shape)
            dtype = mybir.dt.np(alloc.dtype)
            out_names.append(name)
            out_avals.append(jax.core.ShapedArray(shape, dtype))
            zero_shapes.append((shape, dtype))
    n_params = len(in_names)
    n_outs = len(out_names)
    all_names = in_names + out_names
    if partition_name is not None:
        all_names = all_names + [partition_name]
    donate = tuple(range(n_params, n_params + n_outs))

    def _body(*args):
        operands = list(args)
        if partition_name is not None:
            operands.append(bass2jax.partition_id_tensor())
        outs = bass2jax._bass_exec_p.bind(
            *operands,
            out_avals=tuple(out_avals),
            in_names=tuple(all_names),
            out_names=tuple(out_names),
            lowering_input_output_aliases=(),
            sim_require_finite=True,
            sim_require_nnan=True,
            nc=nc,
        )
        return tuple(outs)

    devices = jax.devices()[:NCORES]
    mesh = Mesh(np.asarray(devices), ("core",))
    in_specs = (PartitionSpec("core"),) * (n_params + n_outs)
    out_specs = (PartitionSpec("core"),) * n_outs
    sharded = jax.jit(
        shard_map(_body, mesh=mesh, in_specs=in_specs, out_specs=out_specs,
                  check_rep=False),
        donate_argnums=donate,
        keep_unused=True,
    )

    def run(in_maps):
        concat_in = [
            np.concatenate([np.asarray(in_maps[c][n]) for c in range(NCORES)],
                           axis=0)
            for n in in_names
        ]
        zeros = [np.zeros((NCORES * s[0], *s[1:]), d) for s, d in zero_shapes]
        out_arrs = sharded(*concat_in, *zeros)
        return [
            {
                n: np.asarray(out_arrs[i]).reshape(
                    NCORES, *out_avals[i].shape)[c]
                for i, n in enumerate(out_names)
            }
            for c in range(NCORES)
        ]

    runner = {
        "run": run,
        "sharded": sharded,
        "in_names": in_names,
        "out_names": out_names,
        "out_avals": out_avals,
        "zero_shapes": zero_shapes,
        "mesh": mesh,
    }
    _NC_CACHE["runner"] = runner
    return runner


def kernel(**inputs) -> np.ndarray:
    in_maps = _prep_host(**inputs)
    results = get_runner()["run"](in_maps)
    out = np.empty((B, S, D), np.float32)
    for c in range(NCORES):
        b, hf = divmod(c, 2)
        out[b, hf * SQ:(hf + 1) * SQ, :] = results[c]["out_t"].T
    return out


# revision 20
# speedup vs baseline: 1.1857x; 1.1012x over previous
"""Self-contained 8-core Trainium2 Bass kernel for nn_MultiHeadAttention.

Full (unsharded) inputs in, full output out. Sharding: core c handles
batch b = c // 2, query-half h = c % 2 (1024 queries). K/V projections for a
batch are computed redundantly on the 2 cores sharing it -> zero collectives,
disjoint outputs.

Pipeline (v3):
 - All loads host-packed so each tensor arrives in 1-8 large DMAs.
 - K/V SBUF-resident bf16; no DRAM scratch round trip.
 - PV computed transposed (stationary = probs [keys, q], moving = V[keys, dk]
   + fused ones column): full 128-wide PE output, denominator lands on the
   same partition as its row. PSUM accumulators are memset once and all PV
   matmuls accumulate (start=True would reset the whole shared bank).
 - K projection per head-pair and the previous unit's normalize/transpose
   chains are woven into each attention unit's score/exp slots, keeping the
   Activation engine's exp stream dense.
 - Out-projection for the first query half woven into the last pair.
"""

import ml_dtypes
import numpy as np

import concourse.bass as bass
import concourse.mybir as mybir
from concourse import bacc
from concourse.tile import TileContext
from concourse.bass_utils import run_bass_kernel_spmd

F32 = mybir.dt.float32
F32R = mybir.dt.float32r
BF16 = mybir.dt.bfloat16
ACT = mybir.ActivationFunctionType

B, S, D = 4, 2048, 1024
H, DK = 16, 64
SQ = S // 2            # queries per core
P = 128
NCORES = 8
FC = D // P            # 8 contraction chunks
OFC = D // P           # 8 output-feature chunks
PAIRS = H // 2         # 8 head pairs (2 heads = 128 partitions)
NKT = S // P           # 16 key tiles of 128 tokens
QTW = 512              # q tile width
NQT = SQ // QTW        # 2
NQB = QTW // P         # 4 q-blocks of 128 per q tile
VKG = 256              # v-projection key group
SCALE = 1.0 / np.sqrt(np.float32(DK))
LAG = 2                # PV lags scores/exp by this many kt-pair slots
KW = 256               # k-projection / out-projection chunk width


def build_nc():
    nc = bacc.Bacc()

    xq = nc.declare_dram_parameter("xq_p", [P, FC, SQ], BF16, isOutput=False)
    xkb = nc.declare_dram_parameter("xk_p", [P, FC, S], BF16, isOutput=False)
    xv = nc.declare_dram_parameter("xv_p", [S // VKG, P, FC, VKG], BF16,
                                   isOutput=False)
    wq = nc.declare_dram_parameter("wq_p", [OFC, P, FC, P], BF16,
                                   isOutput=False)
    wkb = nc.declare_dram_parameter("wk_p", [P, FC, D], BF16, isOutput=False)
    wv = nc.declare_dram_parameter("wv_p", [P, FC, D], BF16, isOutput=False)
    wo = nc.declare_dram_parameter("wo_p", [P, PAIRS, D], BF16, isOutput=False)
    bq = nc.declare_dram_parameter("b_q_r", [P, OFC], F32, isOutput=False)
    bk = nc.declare_dram_parameter("b_k_r", [P, OFC], F32, isOutput=False)
    bo = nc.declare_dram_parameter("b_o_r", [P, OFC], F32, isOutput=False)
    idn = nc.declare_dram_parameter("ident", [P, P], BF16, isOutput=False)
    out = nc.declare_dram_parameter("out_t", [D, SQ], F32, isOutput=True)

    with nc.allow_low_precision(reason="bf16 attention"), TileContext(nc) as tc:
        with tc.tile_pool(name="pers", bufs=1) as pers:
            xk_s = pers.tile([P, FC, S], BF16, tag="xk")
            wk_s = pers.tile([P, FC, D], BF16, tag="wk")
            qt_s = pers.tile([P, OFC, SQ], BF16, tag="qt")
            v_all = pers.tile([P, NKT, H, DK + 1], BF16, tag="vall")
            tbq = pers.tile([P, OFC], F32, tag="tbq")
            tbk = pers.tile([P, OFC], F32, tag="tbk")
            tbo = pers.tile([P, OFC], F32, tag="tbo")
            ident = pers.tile([P, P], BF16, tag="ident")
            nc.sync.dma_start(tbq[:], bq[:])
            nc.sync.dma_start(tbk[:], bk[:])
            nc.sync.dma_start(tbo[:], bo[:])
            nc.sync.dma_start(ident[:], idn[:])
            nc.vector.memset(v_all[:, :, :, DK:DK + 1], 1.0)

            # Attention pools that must span stage A (woven first unit)
            ustack = (
                tc.tile_pool(name="kpool", bufs=1),
                tc.tile_pool(name="ptspool", bufs=4),
                tc.tile_pool(name="arawpool", bufs=2),
                tc.tile_pool(name="npool", bufs=2),
                tc.tile_pool(name="spsum", bufs=1, space="PSUM"),
                tc.tile_pool(name="acpsum", bufs=1, space="PSUM"),
            )
            kp, ptsp, arawp, npool, spsum, acpsum = [
                cm.__enter__() for cm in ustack]
            k_all = kp.tile([P, PAIRS, S], BF16, tag="kall")

            # helpers for one (pair, q-tile) attention unit, emitted slotwise
            aux_holder = {}

            def kproj_chunk(c, tt, pool=None):
                pool = pool or aux_holder["auxp"]
                ps = pool.tile([P, QTW], F32, tag=pool._kp_tag,
                               name=f"kp{c}_{tt}")
                tsl = slice(tt * KW, (tt + 1) * KW)
                for fc in range(FC):
                    nc.tensor.matmul(
                        ps[:, 0:KW], wk_s[:, fc, c * P:(c + 1) * P],
                        xk_s[:, fc, tsl],
                        start=(fc == 0), stop=(fc == FC - 1))
                nc.vector.tensor_scalar_add(
                    k_all[:, c, tsl], ps[:, 0:KW], tbk[:, c:c + 1])

            def unit_start(c, qt):
                accs = [acpsum.tile([P, NQB, P], F32, tag=f"acc{h2}",
                                    name=f"acc{c}_{qt}_{h2}")
                        for h2 in range(2)]
                return {"c": c, "qt": qt, "accs": accs, "ptss": {},
                        "qsl": slice(qt * QTW, (qt + 1) * QTW)}

            def unit_slot(st, i):
                c, qt, qsl = st["c"], st["qt"], st["qsl"]
                for h2 in range(2):
                    base = h2 * DK
                    sps = spsum.tile(
                        [P, 2, QTW], F32, tag=f"sps{h2}",
                        name=f"sps{c}_{qt}_{i}_{h2}")
                    for e in range(2):
                        kt = 2 * i + e
                        nc.tensor.matmul(
                            sps[:, e, :],
                            k_all[base:base + DK, c, kt * P:(kt + 1) * P],
                            qt_s[base:base + DK, c, qsl],
                            start=True, stop=True,
                            tile_position=(base, 0))
                    pt = ptsp.tile([P, 2, QTW], BF16, tag=f"pt{h2}",
                                   name=f"pt{c}_{qt}_{i}_{h2}")
                    nc.scalar.activation(pt[:], sps[:], ACT.Exp,
                                         scale=float(SCALE))
                    st["ptss"][(i, h2)] = pt
                if i >= LAG:
                    unit_pv(st, i - LAG)

            def unit_pv(st, i):
                c = st["c"]
                for h2 in range(2):
                    for e in range(2):
                        kt = 2 * i + e
                        for qb in range(NQB):
                            # first matmul into each PSUM bank uses
                            # start=True (zeroes the whole bank)
                            nc.tensor.matmul(
                                st["accs"][h2][:, qb, 0:DK + 1],
                                st["ptss"][(i, h2)][:, e,
                                                    qb * P:(qb + 1) * P],
                                v_all[:, kt, 2 * c + h2, :],
                                start=(kt == 0 and qb == 0 and e == 0),
                                stop=(kt == NKT - 1),
                                skip_group_check=True)

            def unit_finish(st, tail_posts=False):
                c, qt = st["c"], st["qt"]
                for i in range(NKT // 2 - LAG, NKT // 2):
                    unit_pv(st, i)
                araws = []
                for h2 in range(2):
                    araw = arawp.tile([P, NQB, DK + 1], F32, tag="araw",
                                      name=f"araw{c}_{qt}_{h2}")
                    nc.vector.tensor_copy(araw[:],
                                          st["accs"][h2][:, :, 0:DK + 1])
                    araws.append(araw)
                anorms = {}

                def make_post_a(h2, qb):
                    def post_a():
                        araw = araws[h2]
                        recip = npool.tile([P, 1], F32, tag="recip")
                        nc.vector.reciprocal(recip[:],
                                             araw[:, qb, DK:DK + 1])
                        anorm = npool.tile([P, DK], BF16, tag="anorm",
                                           bufs=8)
                        if tail_posts:
                            nc.scalar.mul(anorm[:], araw[:, qb, 0:DK],
                                          recip[:])
                        else:
                            nc.vector.tensor_scalar_mul(
                                anorm[:], araw[:, qb, 0:DK], recip[:])
                        anorms[(h2, qb)] = anorm
                    return post_a

                def make_post_b(h2, qb):
                    def post_b():
                        base = h2 * DK
                        auxp = aux_holder["auxp"]
                        aux = auxp.tile([P, QTW], F32, tag="aux",
                                        name=f"tp{c}_{qt}_{h2}_{qb}")
                        tp = aux[0:DK, 0:DK].bitcast(BF16)
                        nc.tensor.transpose(tp, anorms[(h2, qb)][:],
                                            ident[:])
                        dst = attn_holder["attn_t"][
                            base:base + DK, c,
                            qt * QTW + qb * P:qt * QTW + (qb + 1) * P]
                        if tail_posts:
                            nc.scalar.copy(dst, tp)
                        else:
                            nc.vector.tensor_copy(dst, tp)
                    return post_b

                order = [(h2, qb) for qb in range(NQB) for h2 in range(2)]
                return ([make_post_a(h2, qb) for h2, qb in order]
                        + [make_post_b(h2, qb) for h2, qb in order])

            attn_holder = {}

            # ---------------- Stage A: Q + V projections ----------------
            # Pools opened together so V loads prefetch during Q compute.
            # The first attention unit (pair 0, qt 0) is woven into the
            # later iterations so the Activation engine starts early.
            with (
                tc.tile_pool(name="xqpool", bufs=1) as xqp,
                tc.tile_pool(name="wqpool", bufs=2) as wqp,
                tc.tile_pool(name="wvpool", bufs=1) as wvp,
                tc.tile_pool(name="xvpool", bufs=2) as xvp,
                tc.tile_pool(name="apsum", bufs=2, space="PSUM") as apsum,
            ):
                apsum._kp_tag = "aps"
                xq_t = xqp.tile([P, FC, SQ], BF16, tag="xq")
                # (xq halves DMA'd separately so ofc-0/qt-0 compute starts
                # after half the transfer)

                def load_wq(ofc):
                    wqt = wqp.tile([P, FC, P], BF16, tag="wq",
                                   name=f"wq{ofc}")
                    nc.sync.dma_start(wqt[:], wq[ofc])
                    return wqt

                def load_xv(g):
                    xvt = xvp.tile([P, FC, VKG], BF16, tag="xv",
                                   name=f"xv{g}")
                    nc.sync.dma_start(xvt[:], xv[g])
                    return xvt

                wq_cur = load_wq(0)
                nc.sync.dma_start(xq_t[:, :, 0:QTW], xq[:, :, 0:QTW])
                wvt = wvp.tile([P, FC, D], BF16, tag="wv")
                nc.sync.dma_start(wvt[:, :, 0:QTW], wv[:, :, 0:QTW])
                xv_cur = load_xv(0)
                nc.sync.dma_start(xq_t[:, :, QTW:], xq[:, :, QTW:])
                nc.sync.dma_start(wvt[:, :, QTW:], wv[:, :, QTW:])

                def qproj(ofc, wqt):
                    for qt in range(NQT):
                        qsl = slice(qt * QTW, (qt + 1) * QTW)
                        ps = apsum.tile([P, QTW], F32, tag="aps")
                        for fc in range(FC):
                            nc.tensor.matmul(
                                ps[:], wqt[:, fc, :], xq_t[:, fc, qsl],
                                start=(fc == 0), stop=(fc == FC - 1))
                        nc.vector.tensor_scalar_add(
                            qt_s[:, ofc, qsl], ps[:], tbq[:, ofc:ofc + 1])

                def vproj(g, xvt):
                    for half in range(2):
                        for ki in range(VKG // P):
                            kt = (g * VKG) // P + ki
                            ps = apsum.tile([P, QTW], F32, tag="aps")
                            for fc in range(FC):
                                nc.tensor.matmul(
                                    ps[:],
                                    xvt[:, fc, ki * P:(ki + 1) * P],
                                    wvt[:, fc, half * QTW:(half + 1) * QTW],
                                    start=(fc == 0), stop=(fc == FC - 1))
                            nc.vector.tensor_copy(
                                v_all[:, kt, half * 8:(half + 1) * 8, 0:DK],
                                ps[:].rearrange("p (h d) -> p h d", h=8))

                st0 = None
                slot0 = 0
                slot_plan = {3: 2, 4: 2, 5: 2, 6: 1, 7: 1}
                for ofc in range(OFC):
                    wq_nxt = load_wq(ofc + 1) if ofc + 1 < OFC else None
                    qproj(ofc, wq_cur)
                    wq_cur = wq_nxt
                    xv_nxt = load_xv(ofc + 1) if ofc + 1 < OFC else None
                    vproj(ofc, xv_cur)
                    xv_cur = xv_nxt
                    if ofc == 0:
                        # stage-B inputs ride behind the stage-A stream
                        nc.sync.dma_start(xk_s[:], xkb[:])
                        nc.sync.dma_start(wk_s[:], wkb[:])
                    elif ofc == 1:
                        for tt in range(4):
                            kproj_chunk(0, tt, pool=apsum)
                    elif ofc == 2:
                        for tt in range(4, 8):
                            kproj_chunk(0, tt, pool=apsum)
                        st0 = unit_start(0, 0)
                    else:
                        for _ in range(slot_plan[ofc]):
                            unit_slot(st0, slot0)
                            slot0 += 1
                while slot0 < NKT // 2:
                    unit_slot(st0, slot0)
                    slot0 += 1
                pending0 = unit_finish(st0)

            # ---------------- Stage B: woven attention ----------------
            with (
                tc.tile_pool(name="attnpool", bufs=1) as katp,
                tc.tile_pool(name="opool", bufs=2) as opool,
                tc.tile_pool(name="auxpsum", bufs=2, space="PSUM") as auxp,
            ):
                auxp._kp_tag = "aux"
                aux_holder["auxp"] = auxp
                attn_t = katp.tile([P, PAIRS, SQ], BF16, tag="attnt")
                attn_holder["attn_t"] = attn_t
                wto = katp.tile([P, PAIRS, D], BF16, tag="wo")
                nc.sync.dma_start(wto[:], wo[:])

                def c_chunk(qt, ofc, half, tail=0, dve=False):
                    qsl = slice(qt * QTW + half * KW,
                                qt * QTW + (half + 1) * KW)
                    if tail == 0:
                        ps = auxp.tile([P, QTW], F32, tag="aux",
                                       name=f"cc{qt}_{ofc}_{half}")
                    elif tail == 1:
                        ps = spsum.tile([P, 2, QTW], F32, tag="sps0",
                                        name=f"cc{qt}_{ofc}_{half}")[:, 0, :]
                    else:
                        ps = spsum.tile([P, 2, QTW], F32, tag="sps1",
                                        name=f"cc{qt}_{ofc}_{half}")[:, 0, :]
                    for cc in range(PAIRS):
                        nc.tensor.matmul(
                            ps[:, 0:KW], wto[:, cc, ofc * P:(ofc + 1) * P],
                            attn_t[:, cc, qsl],
                            start=(cc == 0), stop=(cc == PAIRS - 1))
                    osb = opool.tile([P, KW], F32, tag="osb", bufs=4)
                    if tail and not dve:
                        nc.scalar.activation(osb[:], ps[:, 0:KW],
                                             ACT.Identity,
                                             bias=tbo[:, ofc:ofc + 1])
                    else:
                        nc.vector.tensor_scalar_add(osb[:], ps[:, 0:KW],
                                                    tbo[:, ofc:ofc + 1])
                    nc.sync.dma_start(
                        out[ofc * P:(ofc + 1) * P, qsl], osb[:])

                def attn_unit(c, qt, fillers, pending, tail_posts=False):
                    """Emit one (pair, q-tile) attention unit. `pending` are
                    the previous unit's normalize/transpose chains, drained in
                    the early slots; returns this unit's chains."""
                    qsl = slice(qt * QTW, (qt + 1) * QTW)
                    filler_start = 5 if (c == PAIRS - 1 and qt == 1) else 3
                    accs = [acpsum.tile([P, NQB, P], F32, tag=f"acc{h2}",
                                        name=f"acc{c}_{qt}_{h2}")
                            for h2 in range(2)]
                    ptss = {}

                    def scores_exp(i):
                        for h2 in range(2):
                            base = h2 * DK
                            sps = spsum.tile(
                                [P, 2, QTW], F32, tag=f"sps{h2}",
                                name=f"sps{c}_{qt}_{i}_{h2}")
                            for e in range(2):
                                kt = 2 * i + e
                                nc.tensor.matmul(
                                    sps[:, e, :],
                                    k_all[base:base + DK, c,
                                          kt * P:(kt + 1) * P],
                                    qt_s[base:base + DK, c, qsl],
                                    start=True, stop=True,
                                    tile_position=(base, 0))
                            pt = ptsp.tile([P, 2, QTW], BF16, tag=f"pt{h2}",
                                           name=f"pt{c}_{qt}_{i}_{h2}")
                            nc.scalar.activation(pt[:], sps[:], ACT.Exp,
                                                 scale=float(SCALE))
                            ptss[(i, h2)] = pt

                    def pv(i):
                        for h2 in range(2):
                            for e in range(2):
                                kt = 2 * i + e
                                for qb in range(NQB):
                                    # the first matmul into each PSUM bank
                                    # uses start=True, which zeroes the whole
                                    # bank; everything after accumulates
                                    nc.tensor.matmul(
                                        accs[h2][:, qb, 0:DK + 1],
                                        ptss[(i, h2)][:, e,
                                                      qb * P:(qb + 1) * P],
                                        v_all[:, kt, 2 * c + h2, :],
                                        start=(kt == 0 and qb == 0 and
                                               e == 0),
                                        stop=(kt == NKT - 1),
                                        skip_group_check=True)

                    if filler_start == 5:
                        pops = [3, 3, 3, 3, 2, 2, 0, 0]
                    else:
                        pops = [3, 3, 2, 2, 2, 2, 1, 1]
                    for i in range(NKT // 2):
                        scores_exp(i)
                        for _ in range(pops[i]):
                            if pending:
                                pending.pop(0)()
                        if i >= LAG:
                            pv(i - LAG)
                        nf = 1
                        if filler_start == 5:
                            nf = 2
                            filler_start = 6
                        if i >= filler_start:
                            for _ in range(nf):
                                if fillers:
                                    fillers.pop(0)()
                    for i in range(NKT // 2 - LAG, NKT // 2):
                        pv(i)
                    while fillers:
                        fillers.pop(0)()

                    # spill accumulators to SBUF so the PSUM bank frees
                    # immediately and the chains can drain next unit
                    araws = []
                    for h2 in range(2):
                        araw = arawp.tile([P, NQB, DK + 1], F32, tag="araw",
                                          name=f"araw{c}_{qt}_{h2}")
                        nc.vector.tensor_copy(araw[:],
                                              accs[h2][:, :, 0:DK + 1])
                        araws.append(araw)

                    anorms = {}

                    def make_post_a(h2, qb):
                        def post_a():
                            araw = araws[h2]
                            recip = npool.tile([P, 1], F32, tag="recip")
                            nc.vector.reciprocal(recip[:],
                                                 araw[:, qb, DK:DK + 1])
                            anorm = npool.tile([P, DK], BF16, tag="anorm",
                                               bufs=8)
                            if tail_posts:
                                nc.scalar.mul(anorm[:], araw[:, qb, 0:DK],
                                              recip[:])
                            else:
                                nc.vector.tensor_scalar_mul(
                                    anorm[:], araw[:, qb, 0:DK], recip[:])
                            anorms[(h2, qb)] = anorm
                        return post_a

                    def make_post_b(h2, qb):
                        def post_b():
                            base = h2 * DK
                            aux = auxp.tile([P, QTW], F32, tag="aux",
                                            name=f"tp{c}_{qt}_{h2}_{qb}")
                            tp = aux[0:DK, 0:DK].bitcast(BF16)
                            nc.tensor.transpose(tp, anorms[(h2, qb)][:],
                                                ident[:])
                            dst = attn_t[base:base + DK, c,
                                         qt * QTW + qb * P:
                                         qt * QTW + (qb + 1) * P]
                            if tail_posts:
                                nc.scalar.copy(dst, tp)
                            else:
                                nc.vector.tensor_copy(dst, tp)
                        return post_b

                    order = [(h2, qb) for qb in range(NQB) for h2 in range(2)]
                    return ([make_post_a(h2, qb) for h2, qb in order]
                            + [make_post_b(h2, qb) for h2, qb in order])

                pending = pending0
                units = [(0, 1)] + [(c, qt) for c in range(1, PAIRS)
                                    for qt in range(NQT)]
                for c, qt in units:
                    if c + 1 < PAIRS:
                        lo = 0 if (c, qt) == (0, 1) else 4 * qt
                        fillers = [
                            (lambda c_=c + 1, t_=t: kproj_chunk(c_, t_))
                            for t in range(lo, 4 * qt + 4)
                        ]
                    elif qt == 1:
                        fillers = [
                            (lambda o_=o, hf_=hf: c_chunk(0, o_, hf_))
                            for o in range(2) for hf in range(2)
                        ]
                    else:
                        fillers = []
                    pending = attn_unit(c, qt, fillers, pending,
                                        tail_posts=(c == PAIRS - 1
                                                    and qt == 1))
                for p_ in pending:
                    p_()
                tail_work = ([(0, o, hf) for o in range(2, OFC)
                              for hf in range(2)]
                             + [(1, o, hf) for o in range(OFC)
                                for hf in range(2)])
                for j, (qt_, o_, hf_) in enumerate(tail_work):
                    c_chunk(qt_, o_, hf_, tail=(1 + (j % 3)) if (j % 3) < 2
                            else 0, dve=bool(j % 2))

            for cm in reversed(ustack):
                cm.__exit__(None, None, None)

    nc.finalize()
    return nc


def _prep_host(query, key, value, W_q, b_q, W_k, b_k, W_v, b_v, W_out, b_out):
    """Host-side layout prep (packing / transposes / bias folding). No math
    beyond the b_v fold, which is a 1024x1024 matvec."""
    f32 = np.float32
    bf16 = ml_dtypes.bfloat16
    query = np.asarray(query, f32)
    key = np.asarray(key, f32)
    value = np.asarray(value, f32)
    W_q = np.asarray(W_q, f32)
    W_k = np.asarray(W_k, f32)
    W_v = np.asarray(W_v, f32)
    W_out = np.asarray(W_out, f32)
    b_q = np.asarray(b_q, f32)
    b_k = np.asarray(b_k, f32)
    b_v = np.asarray(b_v, f32)
    b_out = np.asarray(b_out, f32)

    def pack_w(wt, dt):  # [D(in), D(of)] -> [P, FC, D(of)]
        return np.ascontiguousarray(
            wt.reshape(FC, P, D).transpose(1, 0, 2)).astype(dt)

    def pack_x(xt, dt, width, n):  # [D, T] -> [n, P, FC, width]
        return np.ascontiguousarray(
            xt.reshape(FC, P, n, width).transpose(2, 1, 0, 3)).astype(dt)

    common = {
        "wq_p": np.ascontiguousarray(
            W_q.T.reshape(FC, P, OFC, P).transpose(2, 1, 0, 3)).astype(bf16),
        "wk_p": pack_w(W_k.T, bf16),
        "wv_p": pack_w(W_v.T, bf16),
        "wo_p": pack_w(W_out.T, bf16),
        "b_q_r": np.ascontiguousarray(b_q.reshape(OFC, P).T),
        "b_k_r": np.ascontiguousarray(b_k.reshape(OFC, P).T),
        "b_o_r": np.ascontiguousarray(
            (b_out + W_out @ b_v).reshape(OFC, P).T.astype(f32)),
        "ident": np.eye(P, dtype=bf16),
    }
    in_maps = []
    for c in range(NCORES):
        b, hf = divmod(c, 2)
        m = dict(common)
        m["xq_p"] = pack_x(query[b, hf * SQ:(hf + 1) * SQ, :].T, bf16,
                           SQ, 1)[0]
        m["xk_p"] = pack_x(key[b].T, bf16, S, 1)[0]
        m["xv_p"] = pack_x(value[b].T, bf16, VKG, S // VKG)
        in_maps.append(m)
    return in_maps


_NC_CACHE = {}


def get_nc():
    if "nc" not in _NC_CACHE:
        _NC_CACHE["nc"] = build_nc()
    return _NC_CACHE["nc"]


def get_runner():
    """Build (once) a cached jitted SPMD callable over 8 cores.

    Mirrors concourse.bass2jax.run_bass_via_pjrt's multi-core path, but keeps
    the jitted function so repeated calls don't recompile the NEFF.
    """
    if "runner" in _NC_CACHE:
        return _NC_CACHE["runner"]

    import jax
    from jax.experimental.shard_map import shard_map
    from jax.sharding import Mesh, PartitionSpec

    from concourse import bass2jax

    nc = get_nc()
    bass2jax.install_neuronx_cc_hook()
    partition_name = (
        nc.partition_id_tensor.name if nc.partition_id_tensor else None
    )

    in_names, out_names, out_avals, zero_shapes = [], [], [], []
    for alloc in nc.m.functions[0].allocations:
        if not isinstance(alloc, mybir.MemoryLocationSet):
            continue
        name = alloc.memorylocations[0].name
        if alloc.kind == "ExternalInput":
            if name != partition_name:
                in_names.append(name)
        elif alloc.kind == "ExternalOutput":
            shape = tuple(alloc.tensor_shape)
            dtype = mybir.dt.np(alloc.dtype)
            out_names.append(name)
            out_avals.append(jax.core.ShapedArray(shape, dtype))
            zero_shapes.append((shape, dtype))
    n_params = len(in_names)
    n_outs = len(out_names)
    all_names = in_names + out_names
    if partition_name is not None:
        all_names = all_names + [partition_name]
    donate = tuple(range(n_params, n_params + n_outs))

    def _body(*args):
        operands = list(args)
        if partition_name is not None:
            operands.append(bass2jax.partition_id_tensor())
        outs = bass2jax._bass_exec_p.bind(
            *operands,
            out_avals=tuple(out_avals),
            in_names=tuple(all_names),
            out_names=tuple(out_names),
            lowering_input_output_aliases=(),
            sim_require_finite=True,
            sim_require_nnan=True,
            nc=nc,
        )
        return tuple(outs)

    devices = jax.devices()[:NCORES]
    mesh = Mesh(np.asarray(devices), ("core",))
    in_specs = (PartitionSpec("core"),) * (n_params + n_outs)
    out_specs = (PartitionSpec("core"),) * n_outs
    sharded = jax.jit(
        shard_map(_body, mesh=mesh, in_specs=in_specs, out_specs=out_specs,
                  check_rep=False),
        donate_argnums=donate,
        keep_unused=True,
    )

    def run(in_maps):
        concat_in = [
            np.concatenate([np.asarray(in_maps[c][n]) for c in range(NCORES)],
                           axis=0)
            for n in in_names
        ]
        zeros = [np.zeros((NCORES * s[0], *s[1:]), d) for s, d in zero_shapes]
        out_arrs = sharded(*concat_in, *zeros)
        return [
            {
                n: np.asarray(out_arrs[i]).reshape(
                    NCORES, *out_avals[i].shape)[c]
                for i, n in enumerate(out_names)
            }
            for c in range(NCORES)
        ]

    runner = {
        "run": run,
        "sharded": sharded,
        "in_names": in_names,
        "out_names": out_names,
        "out_avals": out_avals,
        "zero_shapes": zero_shapes,
        "mesh": mesh,
    }
    _NC_CACHE["runner"] = runner
    return runner


def kernel(**inputs) -> np.ndarray:
    in_maps = _prep_host(**inputs)
    results = get_runner()["run"](in_maps)
    out = np.empty((B, S, D), np.float32)
    for c in range(NCORES):
        b, hf = divmod(c, 2)
        out[b, hf * SQ:(hf + 1) * SQ, :] = results[c]["out_t"].T
    return out


# revision 25
# speedup vs baseline: 1.1881x; 1.0020x over previous
"""Self-contained 8-core Trainium2 Bass kernel for nn_MultiHeadAttention.

Full (unsharded) inputs in, full output out. Sharding: core c handles
batch b = c // 2, query-half h = c % 2 (1024 queries). K/V projections for a
batch are computed redundantly on the 2 cores sharing it -> zero collectives,
disjoint outputs.

Pipeline (v3):
 - All loads host-packed so each tensor arrives in 1-8 large DMAs.
 - K/V SBUF-resident bf16; no DRAM scratch round trip.
 - PV computed transposed (stationary = probs [keys, q], moving = V[keys, dk]
   + fused ones column): full 128-wide PE output, denominator lands on the
   same partition as its row. PSUM accumulators are memset once and all PV
   matmuls accumulate (start=True would reset the whole shared bank).
 - K projection per head-pair and the previous unit's normalize/transpose
   chains are woven into each attention unit's score/exp slots, keeping the
   Activation engine's exp stream dense.
 - Out-projection for the first query half woven into the last pair.
"""

import ml_dtypes
import numpy as np

import concourse.bass as bass
import concourse.mybir as mybir
from concourse import bacc
from concourse.tile import TileContext
from concourse.bass_utils import run_bass_kernel_spmd

F32 = mybir.dt.float32
F32R = mybir.dt.float32r
BF16 = mybir.dt.bfloat16
ACT = mybir.ActivationFunctionType

B, S, D = 4, 2048, 1024
H, DK = 16, 64
SQ = S // 2            # queries per core
P = 128
NCORES = 8
FC = D // P            # 8 contraction chunks
OFC = D // P           # 8 output-feature chunks
PAIRS = H // 2         # 8 head pairs (2 heads = 128 partitions)
NKT = S // P           # 16 key tiles of 128 tokens
QTW = 512              # q tile width
NQT = SQ // QTW        # 2
NQB = QTW // P         # 4 q-blocks of 128 per q tile
VKG = 256              # v-projection key group
SCALE = 1.0 / np.sqrt(np.float32(DK))
LOG2E = 1.4426950408889634
EXP_A = float(128.0 * LOG2E * SCALE)       # Schraudolph exp in bf16-bit space
EXP_B = float(16256.0 - 366393.0 / 65536.0)
DVE_SLOTS = (3, 5)     # unit slots whose exp runs on DVE (bit-trick approx)
LAG = 2                # PV lags scores/exp by this many kt-pair slots
KW = 256               # k-projection / out-projection chunk width


def build_nc():
    nc = bacc.Bacc()

    xq = nc.declare_dram_parameter("xq_p", [P, FC, SQ], BF16, isOutput=False)
    xkb = nc.declare_dram_parameter("xk_p", [P, FC, S], BF16, isOutput=False)
    xv = nc.declare_dram_parameter("xv_p", [S // VKG, P, FC, VKG], BF16,
                                   isOutput=False)
    wq = nc.declare_dram_parameter("wq_p", [OFC, P, FC, P], BF16,
                                   isOutput=False)
    wkb = nc.declare_dram_parameter("wk_p", [P, FC, D], BF16, isOutput=False)
    wv = nc.declare_dram_parameter("wv_p", [P, FC, D], BF16, isOutput=False)
    wo = nc.declare_dram_parameter("wo_p", [P, PAIRS, D], BF16, isOutput=False)
    bq = nc.declare_dram_parameter("b_q_r", [P, OFC], F32, isOutput=False)
    bk = nc.declare_dram_parameter("b_k_r", [P, OFC], F32, isOutput=False)
    bo = nc.declare_dram_parameter("b_o_r", [P, OFC], F32, isOutput=False)
    idn = nc.declare_dram_parameter("ident", [P, P], BF16, isOutput=False)
    out = nc.declare_dram_parameter("out_t", [D, SQ], F32, isOutput=True)

    with nc.allow_low_precision(reason="bf16 attention"), TileContext(nc) as tc:
        with tc.tile_pool(name="pers", bufs=1) as pers:
            xk_s = pers.tile([P, FC, S], BF16, tag="xk")
            wk_s = pers.tile([P, FC, D], BF16, tag="wk")
            qt_s = pers.tile([P, OFC, SQ], BF16, tag="qt")
            v_all = pers.tile([P, NKT, H, DK + 1], BF16, tag="vall")
            tbq = pers.tile([P, OFC], F32, tag="tbq")
            tbk = pers.tile([P, OFC], F32, tag="tbk")
            tbo = pers.tile([P, OFC], F32, tag="tbo")
            ident = pers.tile([P, P], BF16, tag="ident")
            nc.sync.dma_start(tbq[:], bq[:])
            nc.sync.dma_start(tbk[:], bk[:])
            nc.sync.dma_start(tbo[:], bo[:])
            nc.sync.dma_start(ident[:], idn[:])
            nc.vector.memset(v_all[:, :, :, DK:DK + 1], 1.0)

            # Attention pools that must span stage A (woven first unit)
            ustack = (
                tc.tile_pool(name="kpool", bufs=1),
                tc.tile_pool(name="ptspool", bufs=4),
                tc.tile_pool(name="arawpool", bufs=2),
                tc.tile_pool(name="npool", bufs=2),
                tc.tile_pool(name="spsum", bufs=1, space="PSUM"),
                tc.tile_pool(name="acpsum", bufs=1, space="PSUM"),
            )
            kp, ptsp, arawp, npool, spsum, acpsum = [
                cm.__enter__() for cm in ustack]
            k_all = kp.tile([P, PAIRS, S], BF16, tag="kall")

            # helpers for one (pair, q-tile) attention unit, emitted slotwise
            aux_holder = {}

            def kproj_chunk(c, tt, pool=None):
                pool = pool or aux_holder["auxp"]
                ps = pool.tile([P, QTW], F32, tag=pool._kp_tag,
                               name=f"kp{c}_{tt}")
                tsl = slice(tt * KW, (tt + 1) * KW)
                for fc in range(FC):
                    nc.tensor.matmul(
                        ps[:, 0:KW], wk_s[:, fc, c * P:(c + 1) * P],
                        xk_s[:, fc, tsl],
                        start=(fc == 0), stop=(fc == FC - 1))
                nc.vector.tensor_scalar_add(
                    k_all[:, c, tsl], ps[:, 0:KW], tbk[:, c:c + 1])

            def unit_start(c, qt):
                accs = [acpsum.tile([P, NQB, P], F32, tag=f"acc{h2}",
                                    name=f"acc{c}_{qt}_{h2}")
                        for h2 in range(2)]
                return {"c": c, "qt": qt, "accs": accs, "ptss": {},
                        "qsl": slice(qt * QTW, (qt + 1) * QTW)}

            def unit_slot(st, i, dve_exp=False):
                c, qt, qsl = st["c"], st["qt"], st["qsl"]
                for h2 in range(2):
                    base = h2 * DK
                    sps = spsum.tile(
                        [P, 2, QTW], F32, tag=f"sps{h2}",
                        name=f"sps{c}_{qt}_{i}_{h2}")
                    for e in range(2):
                        kt = 2 * i + e
                        nc.tensor.matmul(
                            sps[:, e, :],
                            k_all[base:base + DK, c, kt * P:(kt + 1) * P],
                            qt_s[base:base + DK, c, qsl],
                            start=True, stop=True,
                            tile_position=(base, 0))
                    pt = ptsp.tile([P, 2, QTW], BF16, tag=f"pt{h2}",
                                   name=f"pt{c}_{qt}_{i}_{h2}")
                    if dve_exp:
                        # Schraudolph bit-trick exp, directly in bf16 bit
                        # space: exp(s*x) ~= bitcast_bf16(int16(A*x + B));
                        # ~2% rms on these keys' probs
                        nc.vector.tensor_scalar(
                            pt[:].bitcast(mybir.dt.int16), sps[:],
                            EXP_A, EXP_B,
                            mybir.AluOpType.mult, mybir.AluOpType.add)
                    else:
                        nc.scalar.activation(pt[:], sps[:], ACT.Exp,
                                             scale=float(SCALE))
                    st["ptss"][(i, h2)] = pt
                if i >= LAG:
                    unit_pv(st, i - LAG)

            def unit_pv(st, i):
                c = st["c"]
                for h2 in range(2):
                    for e in range(2):
                        kt = 2 * i + e
                        for qb in range(NQB):
                            # first matmul into each PSUM bank uses
                            # start=True (zeroes the whole bank)
                            nc.tensor.matmul(
                                st["accs"][h2][:, qb, 0:DK + 1],
                                st["ptss"][(i, h2)][:, e,
                                                    qb * P:(qb + 1) * P],
                                v_all[:, kt, 2 * c + h2, :],
                                start=(kt == 0 and qb == 0 and e == 0),
                                stop=(kt == NKT - 1),
                                skip_group_check=True)

            def unit_finish(st, tail_posts=False):
                c, qt = st["c"], st["qt"]
                for i in range(NKT // 2 - LAG, NKT // 2):
                    unit_pv(st, i)
                araws = []
                for h2 in range(2):
                    araw = arawp.tile([P, NQB, DK + 1], F32, tag="araw",
                                      name=f"araw{c}_{qt}_{h2}")
                    nc.vector.tensor_copy(araw[:],
                                          st["accs"][h2][:, :, 0:DK + 1])
                    araws.append(araw)
                anorms = {}

                def make_post_a(h2, qb):
                    def post_a():
                        araw = araws[h2]
                        recip = npool.tile([P, 1], F32, tag="recip")
                        nc.vector.reciprocal(recip[:],
                                             araw[:, qb, DK:DK + 1])
                        if qb not in anorms:
                            anorms[qb] = npool.tile(
                                [P, 2, DK], BF16, tag="anorm", bufs=6,
                                name=f"an{c}_{qt}_{qb}")
                        dst = anorms[qb][:, h2, :]
                        if tail_posts:
                            nc.scalar.mul(dst, araw[:, qb, 0:DK], recip[:])
                        else:
                            nc.vector.tensor_scalar_mul(
                                dst, araw[:, qb, 0:DK], recip[:])
                    return post_a

                def make_post_t(qb):
                    def post_t():
                        # [128q, 2*64 hd] -> [128 hd, 128 q] via the DMA
                        # transpose crossbar; PE/DVE untouched
                        q0 = qt * QTW + qb * P
                        nc.sync.dma_start_transpose(
                            attn_holder["attn_t"][:, c, q0:q0 + P],
                            anorms[qb][:].rearrange("p a b -> p (a b)"))
                    return post_t

                posts = []
                for qb in range(NQB):
                    posts.append(make_post_a(0, qb))
                    posts.append(make_post_a(1, qb))
                    posts.append(make_post_t(qb))
                return posts

            attn_holder = {}

            # ---------------- Stage A: Q + V projections ----------------
            # Pools opened together so V loads prefetch during Q compute.
            # The first attention unit (pair 0, qt 0) is woven into the
            # later iterations so the Activation engine starts early.
            with (
                tc.tile_pool(name="xqpool", bufs=1) as xqp,
                tc.tile_pool(name="wqpool", bufs=2) as wqp,
                tc.tile_pool(name="wvpool", bufs=1) as wvp,
                tc.tile_pool(name="xvpool", bufs=2) as xvp,
                tc.tile_pool(name="apsum", bufs=2, space="PSUM") as apsum,
            ):
                apsum._kp_tag = "aps"
                xq_t = xqp.tile([P, FC, SQ], BF16, tag="xq")
                # (xq halves DMA'd separately so ofc-0/qt-0 compute starts
                # after half the transfer)

                def load_wq(ofc):
                    wqt = wqp.tile([P, FC, P], BF16, tag="wq",
                                   name=f"wq{ofc}")
                    nc.sync.dma_start(wqt[:], wq[ofc])
                    return wqt

                def load_xv(g):
                    xvt = xvp.tile([P, FC, VKG], BF16, tag="xv",
                                   name=f"xv{g}")
                    nc.sync.dma_start(xvt[:], xv[g])
                    return xvt

                wq_cur = load_wq(0)
                nc.sync.dma_start(xq_t[:, :, 0:QTW], xq[:, :, 0:QTW])
                wvt = wvp.tile([P, FC, D], BF16, tag="wv")
                nc.sync.dma_start(wvt[:, :, 0:QTW], wv[:, :, 0:QTW])
                xv_cur = load_xv(0)
                nc.sync.dma_start(xq_t[:, :, QTW:], xq[:, :, QTW:])
                nc.sync.dma_start(wvt[:, :, QTW:], wv[:, :, QTW:])

                def qproj(ofc, wqt):
                    for qt in range(NQT):
                        qsl = slice(qt * QTW, (qt + 1) * QTW)
                        ps = apsum.tile([P, QTW], F32, tag="aps")
                        for fc in range(FC):
                            nc.tensor.matmul(
                                ps[:], wqt[:, fc, :], xq_t[:, fc, qsl],
                                start=(fc == 0), stop=(fc == FC - 1))
                        nc.vector.tensor_scalar_add(
                            qt_s[:, ofc, qsl], ps[:], tbq[:, ofc:ofc + 1])

                def vproj(g, xvt):
                    for half in range(2):
                        for ki in range(VKG // P):
                            kt = (g * VKG) // P + ki
                            ps = apsum.tile([P, QTW], F32, tag="aps")
                            for fc in range(FC):
                                nc.tensor.matmul(
                                    ps[:],
                                    xvt[:, fc, ki * P:(ki + 1) * P],
                                    wvt[:, fc, half * QTW:(half + 1) * QTW],
                                    start=(fc == 0), stop=(fc == FC - 1))
                            nc.vector.tensor_copy(
                                v_all[:, kt, half * 8:(half + 1) * 8, 0:DK],
                                ps[:].rearrange("p (h d) -> p h d", h=8))

                st0 = None
                slot0 = 0
                slot_plan = {3: 1, 4: 2, 5: 2, 6: 2, 7: 1}
                for ofc in range(OFC):
                    if ofc >= 3:
                        for _ in range(slot_plan[ofc]):
                            unit_slot(st0, slot0)
                            slot0 += 1
                    wq_nxt = load_wq(ofc + 1) if ofc + 1 < OFC else None
                    qproj(ofc, wq_cur)
                    wq_cur = wq_nxt
                    xv_nxt = load_xv(ofc + 1) if ofc + 1 < OFC else None
                    vproj(ofc, xv_cur)
                    xv_cur = xv_nxt
                    if ofc == 0:
                        # stage-B inputs ride behind the stage-A stream
                        nc.sync.dma_start(xk_s[:, :, 0:S // 2],
                                          xkb[:, :, 0:S // 2])
                        nc.sync.dma_start(wk_s[:], wkb[:])
                    elif ofc == 1:
                        nc.sync.dma_start(xk_s[:, :, S // 2:],
                                          xkb[:, :, S // 2:])
                        for tt in range(4):
                            kproj_chunk(0, tt, pool=apsum)
                    elif ofc == 2:
                        for tt in range(4, 8):
                            kproj_chunk(0, tt, pool=apsum)
                        st0 = unit_start(0, 0)
                while slot0 < NKT // 2:
                    unit_slot(st0, slot0)
                    slot0 += 1
                pending0 = unit_finish(st0)

            # ---------------- Stage B: woven attention ----------------
            with (
                tc.tile_pool(name="attnpool", bufs=1) as katp,
                tc.tile_pool(name="opool", bufs=2) as opool,
                tc.tile_pool(name="auxpsum", bufs=2, space="PSUM") as auxp,
            ):
                auxp._kp_tag = "aux"
                aux_holder["auxp"] = auxp
                attn_t = katp.tile([P, PAIRS, SQ], BF16, tag="attnt")
                attn_holder["attn_t"] = attn_t
                wto = katp.tile([P, PAIRS, D], BF16, tag="wo")
                nc.sync.dma_start(wto[:], wo[:])

                def c_chunk(qt, ofc, half, tail=0, dve=False):
                    qsl = slice(qt * QTW + half * KW,
                                qt * QTW + (half + 1) * KW)
                    if tail == 0:
                        ps = auxp.tile([P, QTW], F32, tag="aux",
                                       name=f"cc{qt}_{ofc}_{half}")
                    elif tail == 1:
                        ps = spsum.tile([P, 2, QTW], F32, tag="sps0",
                                        name=f"cc{qt}_{ofc}_{half}")[:, 0, :]
                    else:
                        ps = spsum.tile([P, 2, QTW], F32, tag="sps1",
                                        name=f"cc{qt}_{ofc}_{half}")[:, 0, :]
                    for cc in range(PAIRS):
                        nc.tensor.matmul(
                            ps[:, 0:KW], wto[:, cc, ofc * P:(ofc + 1) * P],
                            attn_t[:, cc, qsl],
                            start=(cc == 0), stop=(cc == PAIRS - 1))
                    osb = opool.tile([P, KW], F32, tag="osb", bufs=4)
                    if tail and not dve:
                        nc.scalar.activation(osb[:], ps[:, 0:KW],
                                             ACT.Identity,
                                             bias=tbo[:, ofc:ofc + 1])
                    else:
                        nc.vector.tensor_scalar_add(osb[:], ps[:, 0:KW],
                                                    tbo[:, ofc:ofc + 1])
                    nc.sync.dma_start(
                        out[ofc * P:(ofc + 1) * P, qsl], osb[:])

                def attn_unit(c, qt, fillers, pending, tail_posts=False):
                    """Emit one (pair, q-tile) attention unit. `pending` are
                    the previous unit's normalize/transpose chains, drained in
                    the early slots; returns this unit's chains."""
                    filler_start = 5 if (c == PAIRS - 1 and qt == 1) else 3
                    st = unit_start(c, qt)
                    pops = [3, 3, 3, 3, 0, 0, 0, 0]
                    for i in range(NKT // 2):
                        unit_slot(st, i, dve_exp=(i in DVE_SLOTS))
                        for _ in range(pops[i]):
                            if pending:
                                pending.pop(0)()
                        nf = 1
                        if filler_start == 5:
                            nf = 2
                            filler_start = 6
                        if filler_start == 6:
                            nf = 2
                        if i >= filler_start:
                            for _ in range(nf):
                                if fillers:
                                    fillers.pop(0)()
                    while fillers:
                        fillers.pop(0)()
                    return unit_finish(st, tail_posts=tail_posts)

                pending = pending0
                units = [(0, 1)] + [(c, qt) for c in range(1, PAIRS)
                                    for qt in range(NQT)]
                for c, qt in units:
                    if c + 1 < PAIRS:
                        lo = 0 if (c, qt) == (0, 1) else 4 * qt
                        fillers = [
                            (lambda c_=c + 1, t_=t: kproj_chunk(c_, t_))
                            for t in range(lo, 4 * qt + 4)
                        ]
                    elif qt == 1:
                        fillers = [
                            (lambda o_=o, hf_=hf: c_chunk(0, o_, hf_))
                            for o in range(4) for hf in range(2)
                        ]
                    else:
                        fillers = []
                    pending = attn_unit(c, qt, fillers, pending,
                                        tail_posts=(c == PAIRS - 1
                                                    and qt == 1))
                for p_ in pending:
                    p_()
                tail_work = ([(0, o, hf) for o in range(4, OFC)
                              for hf in range(2)]
                             + [(1, o, hf) for o in range(OFC)
                                for hf in range(2)])
                for j, (qt_, o_, hf_) in enumerate(tail_work):
                    c_chunk(qt_, o_, hf_, tail=(1 + (j % 3)) if (j % 3) < 2
                            else 0, dve=bool(j % 2))

            for cm in reversed(ustack):
                cm.__exit__(None, None, None)

    nc.finalize()
    return nc


def _prep_host(query, key, value, W_q, b_q, W_k, b_k, W_v, b_v, W_out, b_out):
    """Host-side layout prep (packing / transposes / bias folding). No math
    beyond the b_v fold, which is a 1024x1024 matvec."""
    f32 = np.float32
    bf16 = ml_dtypes.bfloat16
    query = np.asarray(query, f32)
    key = np.asarray(key, f32)
    value = np.asarray(value, f32)
    W_q = np.asarray(W_q, f32)
    W_k = np.asarray(W_k, f32)
    W_v = np.asarray(W_v, f32)
    W_out = np.asarray(W_out, f32)
    b_q = np.asarray(b_q, f32)
    b_k = np.asarray(b_k, f32)
    b_v = np.asarray(b_v, f32)
    b_out = np.asarray(b_out, f32)

    def pack_w(wt, dt):  # [D(in), D(of)] -> [P, FC, D(of)]
        return np.ascontiguousarray(
            wt.reshape(FC, P, D).transpose(1, 0, 2)).astype(dt)

    def pack_x(xt, dt, width, n):  # [D, T] -> [n, P, FC, width]
        return np.ascontiguousarray(
            xt.reshape(FC, P, n, width).transpose(2, 1, 0, 3)).astype(dt)

    common = {
        "wq_p": np.ascontiguousarray(
            W_q.T.reshape(FC, P, OFC, P).transpose(2, 1, 0, 3)).astype(bf16),
        "wk_p": pack_w(W_k.T, bf16),
        "wv_p": pack_w(W_v.T, bf16),
        "wo_p": pack_w(W_out.T, bf16),
        "b_q_r": np.ascontiguousarray(b_q.reshape(OFC, P).T),
        "b_k_r": np.ascontiguousarray(b_k.reshape(OFC, P).T),
        "b_o_r": np.ascontiguousarray(
            (b_out + W_out @ b_v).reshape(OFC, P).T.astype(f32)),
        "ident": np.eye(P, dtype=bf16),
    }
    in_maps = []
    for c in range(NCORES):
        b, hf = divmod(c, 2)
        m = dict(common)
        m["xq_p"] = pack_x(query[b, hf * SQ:(hf + 1) * SQ, :].T, bf16,
                           SQ, 1)[0]
        m["xk_p"] = pack_x(key[b].T, bf16, S, 1)[0]
        m["xv_p"] = pack_x(value[b].T, bf16, VKG, S // VKG)
        in_maps.append(m)
    return in_maps


_NC_CACHE = {}


def get_nc():
    if "nc" not in _NC_CACHE:
        _NC_CACHE["nc"] = build_nc()
    return _NC_CACHE["nc"]


def get_runner():
    """Build (once) a cached jitted SPMD callable over 8 cores.

    Mirrors concourse.bass2jax.run_bass_via_pjrt's multi-core path, but keeps
    the jitted function so repeated calls don't recompile the NEFF.
    """
    if "runner" in _NC_CACHE:
        return _NC_CACHE["runner"]

    import jax
    from jax.experimental.shard_map import shard_map
    from jax.sharding import Mesh, PartitionSpec

    from concourse import bass2jax

    nc = get_nc()
    bass2jax.install_neuronx_cc_hook()
    partition_name = (
        nc.partition_id_tensor.name if nc.partition_id_tensor else None
    )

    in_names, out_names, out_avals, zero_shapes = [], [], [], []
    for alloc in nc.m.functions[0].allocations:
        if not isinstance(alloc, mybir.MemoryLocationSet):
            continue
        name = alloc.memorylocations[0].name
        if alloc.kind == "ExternalInput":
            if name != partition_name:
                in_names.append(name)
        elif alloc.kind == "ExternalOutput":
            shape = tuple(alloc.tensor_shape)
            dtype = mybir.dt.np(alloc.dtype)
            out_names.append(name)
            out_avals.append(jax.core.ShapedArray(shape, dtype))
            zero_shapes.append((shape, dtype))
    n_params = len(in_names)
    n_outs = len(out_names)
    all_names = in_names + out_names
    if partition_name is not None:
        all_names = all_names + [partition_name]
    donate = tuple(range(n_params, n_params + n_outs))

    def _body(*args):
        operands = list(args)
        if partition_name is not None:
            operands.append(bass2jax.partition_id_tensor())
        outs = bass2jax._bass_exec_p.bind(
            *operands,
            out_avals=tuple(out_avals),
            in_names=tuple(all_names),
            out_names=tuple(out_names),
            lowering_input_output_aliases=(),
            sim_require_finite=True,
            sim_require_nnan=True,
            nc=nc,
        )
        return tuple(outs)

    devices = jax.devices()[:NCORES]
    mesh = Mesh(np.asarray(devices), ("core",))
    in_specs = (PartitionSpec("core"),) * (n_params + n_outs)
    out_specs = (PartitionSpec("core"),) * n_outs
    sharded = jax.jit(
        shard_map(_body, mesh=mesh, in_specs=in_specs, out_specs=out_specs,
                  check_rep=False),
        donate_argnums=donate,
        keep_unused=True,
    )

    def run(in_maps):
        concat_in = [
            np.concatenate([np.asarray(in_maps[c][n]) for c in range(NCORES)],
                           axis=0)
            for n in in_names
        ]
        zeros = [np.zeros((NCORES * s[0], *s[1:]), d) for s, d in zero_shapes]
        out_arrs = sharded(*concat_in, *zeros)
        return [
            {
                n: np.asarray(out_arrs[i]).reshape(
                    NCORES, *out_avals[i].shape)[c]
                for i, n in enumerate(out_names)
            }
            for c in range(NCORES)
        ]

    runner = {
        "run": run,
        "sharded": sharded,
        "in_names": in_names,
        "out_names": out_names,
        "out_avals": out_avals,
        "zero_shapes": zero_shapes,
        "mesh": mesh,
    }
    _NC_CACHE["runner"] = runner
    return runner


def kernel(**inputs) -> np.ndarray:
    in_maps = _prep_host(**inputs)
    results = get_runner()["run"](in_maps)
    out = np.empty((B, S, D), np.float32)
    for c in range(NCORES):
        b, hf = divmod(c, 2)
        out[b, hf * SQ:(hf + 1) * SQ, :] = results[c]["out_t"].T
    return out


# revision 41
# speedup vs baseline: 1.2274x; 1.0331x over previous
"""Self-contained 8-core Trainium2 Bass kernel for nn_MultiHeadAttention.

Full (unsharded) inputs in, full output out. Sharding: core c handles
batch b = c // 2, query-half h = c % 2 (1024 queries). K/V projections for a
batch are computed redundantly on the 2 cores sharing it -> zero collectives,
disjoint outputs.

Design (TimelineSim 388.7us vs 477.1us baseline):
 - All loads host-packed (bf16) so each tensor arrives in 1-8 large DMAs,
   ordered by first use; K/V/Q/probs SBUF-resident bf16, no DRAM bounce.
 - PV computed transposed (stationary = probs [keys, q], moving = V[keys, dk]
   + fused ones column): full 128-wide PE output and the softmax denominator
   lands on the same partition as its row, so normalize is a per-partition
   DVE multiply. PSUM accumulators share banks; only the first matmul into a
   bank uses start=True (which zeroes the whole bank), everything after
   accumulates with start=False.
 - Attention runs as 16 (pair, q-tile) units of 8 score/exp/PV slots.
   Per-pair K-projection chunks, the previous unit's normalize chains, and
   out-projection chunks are woven into each unit's slots so PE, Act, and
   DVE all stay busy; exp for 3 of 8 slots runs on DVE via a Schraudolph
   bit-trick in bf16 space (~2% rms on those keys) to unload Act.
 - The first unit is woven into stage A (Q/V projections) so the Activation
   engine starts ~60us earlier; the second unit's scores/exp are emitted at
   the stage-A tail into a pool reusing the projection buffers, so the first
   stage-B unit is PV-only. [q, hd] -> [hd, q] layout restoration uses the
   DMA transpose crossbar instead of PE.
"""

import ml_dtypes
import numpy as np

import concourse.bass as bass
import concourse.mybir as mybir
from concourse import bacc
from concourse.tile import TileContext
from concourse.bass_utils import run_bass_kernel_spmd

F32 = mybir.dt.float32
F32R = mybir.dt.float32r
BF16 = mybir.dt.bfloat16
ACT = mybir.ActivationFunctionType

B, S, D = 4, 2048, 1024
H, DK = 16, 64
SQ = S // 2            # queries per core
P = 128
NCORES = 8
FC = D // P            # 8 contraction chunks
OFC = D // P           # 8 output-feature chunks
PAIRS = H // 2         # 8 head pairs (2 heads = 128 partitions)
NKT = S // P           # 16 key tiles of 128 tokens
QTW = 512              # q tile width
NQT = SQ // QTW        # 2
NQB = QTW // P         # 4 q-blocks of 128 per q tile
VKG = 256              # v-projection key group
SCALE = 1.0 / np.sqrt(np.float32(DK))
LOG2E = 1.4426950408889634
EXP_A = float(128.0 * LOG2E * SCALE)       # Schraudolph exp in bf16-bit space
EXP_B = float(16256.0 - 366393.0 / 65536.0)
DVE_SLOTS = (2, 4, 6)  # unit slots whose exp runs on DVE (bit-trick approx)
LAG = 3                # PV lags scores/exp by this many kt-pair slots
KW = 256               # k-projection / out-projection chunk width


def build_nc():
    nc = bacc.Bacc()

    xq = nc.declare_dram_parameter("xq_p", [P, FC, SQ], BF16, isOutput=False)
    xkb = nc.declare_dram_parameter("xk_p", [P, FC, S], BF16, isOutput=False)
    xv = nc.declare_dram_parameter("xv_p", [S // VKG, P, FC, VKG], BF16,
                                   isOutput=False)
    wq = nc.declare_dram_parameter("wq_p", [OFC, P, FC, P], BF16,
                                   isOutput=False)
    wkb = nc.declare_dram_parameter("wk_p", [P, FC, D], BF16, isOutput=False)
    wv = nc.declare_dram_parameter("wv_p", [P, FC, D], BF16, isOutput=False)
    wo = nc.declare_dram_parameter("wo_p", [P, PAIRS, D], BF16, isOutput=False)
    bq = nc.declare_dram_parameter("b_q_r", [P, OFC], F32, isOutput=False)
    bk = nc.declare_dram_parameter("b_k_r", [P, OFC], F32, isOutput=False)
    bo = nc.declare_dram_parameter("b_o_r", [P, OFC], F32, isOutput=False)
    idn = nc.declare_dram_parameter("ident", [P, P], BF16, isOutput=False)
    out = nc.declare_dram_parameter("out_t", [D, SQ], F32, isOutput=True)

    with nc.allow_low_precision(reason="bf16 attention"), TileContext(nc) as tc:
        with tc.tile_pool(name="pers", bufs=1) as pers:
            xk_s = pers.tile([P, FC, S], BF16, tag="xk")
            wk_s = pers.tile([P, FC, D], BF16, tag="wk")
            qt_s = pers.tile([P, OFC, SQ], BF16, tag="qt")
            v_all = pers.tile([P, NKT, H, DK + 1], BF16, tag="vall")
            tbq = pers.tile([P, OFC], F32, tag="tbq")
            tbk = pers.tile([P, OFC], F32, tag="tbk")
            tbo = pers.tile([P, OFC], F32, tag="tbo")
            ident = pers.tile([P, P], BF16, tag="ident")
            nc.sync.dma_start(tbq[:], bq[:])
            nc.sync.dma_start(tbk[:], bk[:])
            nc.sync.dma_start(tbo[:], bo[:])
            nc.sync.dma_start(ident[:], idn[:])
            nc.vector.memset(v_all[:, :, :, DK:DK + 1], 1.0)

            # Attention pools that must span stage A (woven first unit)
            ustack = (
                tc.tile_pool(name="kpool", bufs=1),
                tc.tile_pool(name="ptspool", bufs=4),
                tc.tile_pool(name="arawpool", bufs=2),
                tc.tile_pool(name="npool", bufs=2),
                tc.tile_pool(name="spsum", bufs=1, space="PSUM"),
                tc.tile_pool(name="acpsum", bufs=1, space="PSUM"),
            )
            kp, ptsp, arawp, npool, spsum, acpsum = [
                cm.__enter__() for cm in ustack]
            k_all = kp.tile([P, PAIRS, S], BF16, tag="kall")

            # helpers for one (pair, q-tile) attention unit, emitted slotwise
            aux_holder = {}

            def kproj_chunk(c, tt, pool=None):
                pool = pool or aux_holder["auxp"]
                ps = pool.tile([P, QTW], F32, tag=pool._kp_tag,
                               name=f"kp{c}_{tt}")
                tsl = slice(tt * KW, (tt + 1) * KW)
                for fc in range(FC):
                    nc.tensor.matmul(
                        ps[:, 0:KW], wk_s[:, fc, c * P:(c + 1) * P],
                        xk_s[:, fc, tsl],
                        start=(fc == 0), stop=(fc == FC - 1))
                nc.vector.tensor_scalar_add(
                    k_all[:, c, tsl], ps[:, 0:KW], tbk[:, c:c + 1])

            def unit_start(c, qt):
                accs = [acpsum.tile([P, NQB, P], F32, tag=f"acc{h2}",
                                    name=f"acc{c}_{qt}_{h2}")
                        for h2 in range(2)]
                return {"c": c, "qt": qt, "accs": accs, "ptss": {},
                        "qsl": slice(qt * QTW, (qt + 1) * QTW)}

            def unit_slot(st, i, dve_exp=False, no_pv=False, pool=None):
                pool = pool or ptsp
                c, qt, qsl = st["c"], st["qt"], st["qsl"]
                for h2 in range(2):
                    base = h2 * DK
                    sps = spsum.tile(
                        [P, 2, QTW], F32, tag=f"sps{h2}",
                        name=f"sps{c}_{qt}_{i}_{h2}")
                    for e in range(2):
                        kt = 2 * i + e
                        nc.tensor.matmul(
                            sps[:, e, :],
                            k_all[base:base + DK, c, kt * P:(kt + 1) * P],
                            qt_s[base:base + DK, c, qsl],
                            start=True, stop=True,
                            tile_position=(base, 0))
                    pt = pool.tile([P, 2, QTW], BF16, tag=f"pt{h2}",
                                   name=f"pt{c}_{qt}_{i}_{h2}")
                    if dve_exp:
                        # Schraudolph bit-trick exp, directly in bf16 bit
                        # space: exp(s*x) ~= bitcast_bf16(int16(A*x + B));
                        # ~2% rms on these keys' probs
                        nc.vector.tensor_scalar(
                            pt[:].bitcast(mybir.dt.int16), sps[:],
                            EXP_A, EXP_B,
                            mybir.AluOpType.mult, mybir.AluOpType.add)
                    else:
                        nc.scalar.activation(pt[:], sps[:], ACT.Exp,
                                             scale=float(SCALE))
                    st["ptss"][(i, h2)] = pt
                if not no_pv and i >= LAG:
                    unit_pv(st, i - LAG)

            def unit_pv(st, i):
                c = st["c"]
                for h2 in range(2):
                    for e in range(2):
                        kt = 2 * i + e
                        for qb in range(NQB):
                            # first matmul into each PSUM bank uses
                            # start=True (zeroes the whole bank)
                            nc.tensor.matmul(
                                st["accs"][h2][:, qb, 0:DK + 1],
                                st["ptss"][(i, h2)][:, e,
                                                    qb * P:(qb + 1) * P],
                                v_all[:, kt, 2 * c + h2, :],
                                start=(kt == 0 and qb == 0 and e == 0),
                                stop=(kt == NKT - 1),
                                skip_group_check=True)

            def unit_finish(st, tail_posts=False):
                c, qt = st["c"], st["qt"]
                for i in range(NKT // 2 - LAG, NKT // 2):
                    unit_pv(st, i)
                araws = []
                for h2 in range(2):
                    araw = arawp.tile([P, NQB, DK + 1], F32, tag="araw",
                                      name=f"araw{c}_{qt}_{h2}")
                    nc.vector.tensor_copy(araw[:],
                                          st["accs"][h2][:, :, 0:DK + 1])
                    araws.append(araw)
                anorms = {}

                def make_post_a(h2, qb):
                    def post_a():
                        araw = araws[h2]
                        recip = npool.tile([P, 1], F32, tag="recip")
                        nc.vector.reciprocal(recip[:],
                                             araw[:, qb, DK:DK + 1])
                        if qb not in anorms:
                            anorms[qb] = npool.tile(
                                [P, 2, DK], BF16, tag="anorm", bufs=6,
                                name=f"an{c}_{qt}_{qb}")
                        dst = anorms[qb][:, h2, :]
                        if tail_posts:
                            nc.scalar.mul(dst, araw[:, qb, 0:DK], recip[:])
                        else:
                            nc.vector.tensor_scalar_mul(
                                dst, araw[:, qb, 0:DK], recip[:])
                    return post_a

                def make_post_t(qb):
                    def post_t():
                        # [128q, 2*64 hd] -> [128 hd, 128 q] via the DMA
                        # transpose crossbar; PE/DVE untouched
                        q0 = qt * QTW + qb * P
                        nc.sync.dma_start_transpose(
                            attn_holder["attn_t"][:, c, q0:q0 + P],
                            anorms[qb][:].rearrange("p a b -> p (a b)"))
                    return post_t

                posts = []
                for qb in range(NQB):
                    posts.append(make_post_a(0, qb))
                    posts.append(make_post_a(1, qb))
                    posts.append(make_post_t(qb))
                return posts

            attn_holder = {}

            # ---------------- Stage A: Q + V projections ----------------
            # Pools opened together so V loads prefetch during Q compute.
            # The first attention unit (pair 0, qt 0) is woven into the
            # later iterations so the Activation engine starts early.
            with (
                tc.tile_pool(name="xqpool", bufs=1) as xqp,
                tc.tile_pool(name="wqpool", bufs=2) as wqp,
                tc.tile_pool(name="wvpool", bufs=1) as wvp,
                tc.tile_pool(name="xvpool", bufs=2) as xvp,
                tc.tile_pool(name="apsum", bufs=2, space="PSUM") as apsum,
            ):
                apsum._kp_tag = "aps"
                xq_t = xqp.tile([P, FC, SQ], BF16, tag="xq")
                # (xq halves DMA'd separately so ofc-0/qt-0 compute starts
                # after half the transfer)

                def load_wq(ofc):
                    wqt = wqp.tile([P, FC, P], BF16, tag="wq",
                                   name=f"wq{ofc}")
                    nc.sync.dma_start(wqt[:], wq[ofc])
                    return wqt

                def load_xv(g):
                    xvt = xvp.tile([P, FC, VKG], BF16, tag="xv",
                                   name=f"xv{g}")
                    nc.sync.dma_start(xvt[:], xv[g])
                    return xvt

                wq_cur = load_wq(0)
                nc.sync.dma_start(xq_t[:, :, 0:QTW], xq[:, :, 0:QTW])
                nc.sync.dma_start(xq_t[:, :, QTW:], xq[:, :, QTW:])
                wq_pre = load_wq(1)
                wvt = wvp.tile([P, FC, D], BF16, tag="wv")
                nc.sync.dma_start(wvt[:, :, 0:QTW], wv[:, :, 0:QTW])
                xv_cur = load_xv(0)
                nc.sync.dma_start(wvt[:, :, QTW:], wv[:, :, QTW:])

                def qproj(ofc, wqt):
                    for qt in range(NQT):
                        qsl = slice(qt * QTW, (qt + 1) * QTW)
                        ps = apsum.tile([P, QTW], F32, tag="aps")
                        for fc in range(FC):
                            nc.tensor.matmul(
                                ps[:], wqt[:, fc, :], xq_t[:, fc, qsl],
                                start=(fc == 0), stop=(fc == FC - 1))
                        nc.vector.tensor_scalar_add(
                            qt_s[:, ofc, qsl], ps[:], tbq[:, ofc:ofc + 1])

                def vproj(g, xvt):
                    for half in range(2):
                        for ki in range(VKG // P):
                            kt = (g * VKG) // P + ki
                            ps = apsum.tile([P, QTW], F32, tag="aps")
                            for fc in range(FC):
                                nc.tensor.matmul(
                                    ps[:],
                                    xvt[:, fc, ki * P:(ki + 1) * P],
                                    wvt[:, fc, half * QTW:(half + 1) * QTW],
                                    start=(fc == 0), stop=(fc == FC - 1))
                            nc.vector.tensor_copy(
                                v_all[:, kt, half * 8:(half + 1) * 8, 0:DK],
                                ps[:].rearrange("p (h d) -> p h d", h=8))

                st0 = None
                slot0 = 0
                slot_plan = {3: 1, 4: 2, 5: 2, 6: 2, 7: 1}
                for ofc in range(OFC):
                    if ofc >= 3:
                        for _ in range(slot_plan[ofc]):
                            unit_slot(st0, slot0)
                            slot0 += 1
                    if ofc == 0:
                        wq_nxt = wq_pre
                    else:
                        wq_nxt = load_wq(ofc + 1) if ofc + 1 < OFC else None
                    qproj(ofc, wq_cur)
                    wq_cur = wq_nxt
                    xv_nxt = load_xv(ofc + 1) if ofc + 1 < OFC else None
                    vproj(ofc, xv_cur)
                    xv_cur = xv_nxt
                    if ofc == 0:
                        # stage-B inputs ride behind the stage-A stream
                        nc.sync.dma_start(xk_s[:, :, 0:S // 2],
                                          xkb[:, :, 0:S // 2])
                        nc.sync.dma_start(wk_s[:], wkb[:])
                    elif ofc == 1:
                        nc.sync.dma_start(xk_s[:, :, S // 2:],
                                          xkb[:, :, S // 2:])
                        for tt in range(4):
                            kproj_chunk(0, tt, pool=apsum)
                    elif ofc == 2:
                        for tt in range(4, 8):
                            kproj_chunk(0, tt, pool=apsum)
                        st0 = unit_start(0, 0)
                while slot0 < NKT // 2:
                    unit_slot(st0, slot0)
                    slot0 += 1
                pending0 = unit_finish(st0)

            # ---------------- Stage B: woven attention ----------------
            with (
                tc.tile_pool(name="attnpool", bufs=1) as katp,
                tc.tile_pool(name="opool", bufs=2) as opool,
                tc.tile_pool(name="auxpsum", bufs=2, space="PSUM") as auxp,
            ):
                auxp._kp_tag = "aux"
                aux_holder["auxp"] = auxp
                attn_t = katp.tile([P, PAIRS, SQ], BF16, tag="attnt")
                attn_holder["attn_t"] = attn_t

                # (0, qt1) scores/exp emitted now, into a pool reusing the
                # closed stage-A space: the first stage-B unit runs PV-only
                # and Act absorbs these exps during the stage-A tail.
                heldp_cm = tc.tile_pool(name="heldp", bufs=8)
                heldp = heldp_cm.__enter__()
                st0b = unit_start(0, 1)
                for i in range(NKT // 2):
                    unit_slot(st0b, i, no_pv=True, pool=heldp)

                def c_chunk(qt, ofc, half, tail=0, dve=False):
                    qsl = slice(qt * QTW + half * KW,
                                qt * QTW + (half + 1) * KW)
                    if tail == 0:
                        ps = auxp.tile([P, QTW], F32, tag="aux",
                                       name=f"cc{qt}_{ofc}_{half}")
                    elif tail == 1:
                        ps = spsum.tile([P, 2, QTW], F32, tag="sps0",
                                        name=f"cc{qt}_{ofc}_{half}")[:, 0, :]
                    else:
                        ps = spsum.tile([P, 2, QTW], F32, tag="sps1",
                                        name=f"cc{qt}_{ofc}_{half}")[:, 0, :]
                    wto = attn_holder["wto"]
                    for cc in range(PAIRS):
                        nc.tensor.matmul(
                            ps[:, 0:KW], wto[:, cc, ofc * P:(ofc + 1) * P],
                            attn_t[:, cc, qsl],
                            start=(cc == 0), stop=(cc == PAIRS - 1))
                    osb = opool.tile([P, KW], F32, tag="osb", bufs=4)
                    if tail and not dve:
                        nc.scalar.activation(osb[:], ps[:, 0:KW],
                                             ACT.Identity,
                                             bias=tbo[:, ofc:ofc + 1])
                    else:
                        nc.vector.tensor_scalar_add(osb[:], ps[:, 0:KW],
                                                    tbo[:, ofc:ofc + 1])
                    nc.sync.dma_start(
                        out[ofc * P:(ofc + 1) * P, qsl], osb[:])

                def attn_unit(c, qt, fillers, pending, tail_posts=False,
                              pre_st=None):
                    """Emit one (pair, q-tile) attention unit. `pending` are
                    the previous unit's normalize/transpose chains, drained in
                    the early slots; returns this unit's chains."""
                    filler_start = 5 if (c == PAIRS - 1 and qt == 1) else 3
                    st = pre_st if pre_st is not None else unit_start(c, qt)
                    pops = [3, 3, 3, 3, 0, 0, 0, 0]
                    for i in range(NKT // 2):
                        if pre_st is None:
                            unit_slot(st, i, dve_exp=(i in DVE_SLOTS))
                        elif i >= LAG:
                            unit_pv(st, i - LAG)
                        for _ in range(pops[i]):
                            if pending:
                                pending.pop(0)()
                        nf = 2
                        if filler_start == 5:
                            filler_start = 6
                        if i >= filler_start:
                            for _ in range(nf):
                                if fillers:
                                    fillers.pop(0)()
                    while fillers:
                        fillers.pop(0)()
                    return unit_finish(st, tail_posts=tail_posts)

                pending = pending0
                units = [(0, 1)] + [(c, qt) for c in range(1, PAIRS)
                                    for qt in range(NQT)]
                for c, qt in units:
                    pre_st = st0b if (c, qt) == (0, 1) else None
                    if c + 1 < PAIRS:
                        lo = 0 if (c, qt) == (0, 1) else 4 * qt
                        fillers = [
                            (lambda c_=c + 1, t_=t: kproj_chunk(c_, t_))
                            for t in range(lo, 4 * qt + 4)
                        ]
                    elif qt == 1:
                        fillers = [
                            (lambda o_=o, hf_=hf: c_chunk(0, o_, hf_))
                            for o in range(4) for hf in range(2)
                        ]
                    else:
                        fillers = []
                    pending = attn_unit(c, qt, fillers, pending,
                                        tail_posts=(c == PAIRS - 1
                                                    and qt == 1),
                                        pre_st=pre_st)
                    if (c, qt) == (0, 1):
                        heldp_cm.__exit__(None, None, None)
                        wtop_cm = tc.tile_pool(name="wtopool", bufs=1)
                        wtop = wtop_cm.__enter__()
                        wto_t = wtop.tile([P, PAIRS, D], BF16, tag="wo")
                        nc.sync.dma_start(wto_t[:], wo[:])
                        attn_holder["wto"] = wto_t
                for p_ in pending:
                    p_()
                tail_work = ([(0, o, hf) for o in range(4, OFC)
                              for hf in range(2)]
                             + [(1, o, hf) for o in range(OFC)
                                for hf in range(2)])
                for j, (qt_, o_, hf_) in enumerate(tail_work):
                    c_chunk(qt_, o_, hf_, tail=(1 + (j % 3)) if (j % 3) < 2
                            else 0, dve=bool(j % 2))

                wtop_cm.__exit__(None, None, None)
            for cm in reversed(ustack):
                cm.__exit__(None, None, None)

    nc.finalize()
    return nc


def _prep_host(query, key, value, W_q, b_q, W_k, b_k, W_v, b_v, W_out, b_out):
    """Host-side layout prep (packing / transposes / bias folding). No math
    beyond the b_v fold, which is a 1024x1024 matvec."""
    f32 = np.float32
    bf16 = ml_dtypes.bfloat16
    query = np.asarray(query, f32)
    key = np.asarray(key, f32)
    value = np.asarray(value, f32)
    W_q = np.asarray(W_q, f32)
    W_k = np.asarray(W_k, f32)
    W_v = np.asarray(W_v, f32)
    W_out = np.asarray(W_out, f32)
    b_q = np.asarray(b_q, f32)
    b_k = np.asarray(b_k, f32)
    b_v = np.asarray(b_v, f32)
    b_out = np.asarray(b_out, f32)

    def pack_w(wt, dt):  # [D(in), D(of)] -> [P, FC, D(of)]
        return np.ascontiguousarray(
            wt.reshape(FC, P, D).transpose(1, 0, 2)).astype(dt)

    def pack_x(xt, dt, width, n):  # [D, T] -> [n, P, FC, width]
        return np.ascontiguousarray(
            xt.reshape(FC, P, n, width).transpose(2, 1, 0, 3)).astype(dt)

    common = {
        "wq_p": np.ascontiguousarray(
            W_q.T.reshape(FC, P, OFC, P).transpose(2, 1, 0, 3)).astype(bf16),
        "wk_p": pack_w(W_k.T, bf16),
        "wv_p": pack_w(W_v.T, bf16),
        "wo_p": pack_w(W_out.T, bf16),
        "b_q_r": np.ascontiguousarray(b_q.reshape(OFC, P).T),
        "b_k_r": np.ascontiguousarray(b_k.reshape(OFC, P).T),
        "b_o_r": np.ascontiguousarray(
            (b_out + W_out @ b_v).reshape(OFC, P).T.astype(f32)),
        "ident": np.eye(P, dtype=bf16),
    }
    in_maps = []
    for c in range(NCORES):
        b, hf = divmod(c, 2)
        m = dict(common)
        m["xq_p"] = pack_x(query[b, hf * SQ:(hf + 1) * SQ, :].T, bf16,
                           SQ, 1)[0]
        m["xk_p"] = pack_x(key[b].T, bf16, S, 1)[0]
        m["xv_p"] = pack_x(value[b].T, bf16, VKG, S // VKG)
        in_maps.append(m)
    return in_maps


_NC_CACHE = {}


def get_nc():
    if "nc" not in _NC_CACHE:
        _NC_CACHE["nc"] = build_nc()
    return _NC_CACHE["nc"]


def get_runner():
    """Build (once) a cached jitted SPMD callable over 8 cores.

    Mirrors concourse.bass2jax.run_bass_via_pjrt's multi-core path, but keeps
    the jitted function so repeated calls don't recompile the NEFF.
    """
    if "runner" in _NC_CACHE:
        return _NC_CACHE["runner"]

    import jax
    from jax.experimental.shard_map import shard_map
    from jax.sharding import Mesh, PartitionSpec

    from concourse import bass2jax

    nc = get_nc()
    bass2jax.install_neuronx_cc_hook()
    partition_name = (
        nc.partition_id_tensor.name if nc.partition_id_tensor else None
    )

    in_names, out_names, out_avals, zero_shapes = [], [], [], []
    for alloc in nc.m.functions[0].allocations:
        if not isinstance(alloc, mybir.MemoryLocationSet):
            continue
        name = alloc.memorylocations[0].name
        if alloc.kind == "ExternalInput":
            if name != partition_name:
                in_names.append(name)
        elif alloc.kind == "ExternalOutput":
            shape = tuple(alloc.tensor_shape)
            dtype = mybir.dt.np(alloc.dtype)
            out_names.append(name)
            out_avals.append(jax.core.ShapedArray(shape, dtype))
            zero_shapes.append((shape, dtype))
    n_params = len(in_names)
    n_outs = len(out_names)
    all_names = in_names + out_names
    if partition_name is not None:
        all_names = all_names + [partition_name]
    donate = tuple(range(n_params, n_params + n_outs))

    def _body(*args):
        operands = list(args)
        if partition_name is not None:
            operands.append(bass2jax.partition_id_tensor())
        outs = bass2jax._bass_exec_p.bind(
            *operands,
            out_avals=tuple(out_avals),
            in_names=tuple(all_names),
            out_names=tuple(out_names),
            lowering_input_output_aliases=(),
            sim_require_finite=True,
            sim_require_nnan=True,
            nc=nc,
        )
        return tuple(outs)

    devices = jax.devices()[:NCORES]
    mesh = Mesh(np.asarray(devices), ("core",))
    in_specs = (PartitionSpec("core"),) * (n_params + n_outs)
    out_specs = (PartitionSpec("core"),) * n_outs
    sharded = jax.jit(
        shard_map(_body, mesh=mesh, in_specs=in_specs, out_specs=out_specs,
                  check_rep=False),
        donate_argnums=donate,
        keep_unused=True,
    )

    def run(in_maps):
        concat_in = [
            np.concatenate([np.asarray(in_maps[c][n]) for c in range(NCORES)],
                           axis=0)
            for n in in_names
        ]
        zeros = [np.zeros((NCORES * s[0], *s[1:]), d) for s, d in zero_shapes]
        out_arrs = sharded(*concat_in, *zeros)
        return [
            {
                n: np.asarray(out_arrs[i]).reshape(
                    NCORES, *out_avals[i].shape)[c]
                for i, n in enumerate(out_names)
            }
            for c in range(NCORES)
        ]

    runner = {
        "run": run,
        "sharded": sharded,
        "in_names": in_names,
        "out_names": out_names,
        "out_avals": out_avals,
        "zero_shapes": zero_shapes,
        "mesh": mesh,
    }
    _NC_CACHE["runner"] = runner
    return runner


def kernel(**inputs) -> np.ndarray:
    in_maps = _prep_host(**inputs)
    results = get_runner()["run"](in_maps)
    out = np.empty((B, S, D), np.float32)
    for c in range(NCORES):
        b, hf = divmod(c, 2)
        out[b, hf * SQ:(hf + 1) * SQ, :] = results[c]["out_t"].T
    return out
